# revision 1
# baseline (speedup 1.0000x reference)
"""Multi-head attention (B=2, S=4096, D=768, H=12, d_k=64) on 8 TRN2 cores.

Sharding: core c -> batch b = c//4, head group g = c%4 (heads 3g..3g+2).
Each core computes partial = sum_{h in group} softmax(QK^T/8) V @ Wo_h^T
over its batch; host sums the 4 partials per batch and adds bo.

Device kernel (identical SPMD program, per-core data):
  Phase A: QKV projections (fp32r matmuls), Q^T/K^T/V^T produced in
           [head_dim, seq] layout (bf16), V transposed to natural
           [seq, head_dim] layout with a ones column appended (row sums).
  Phase B: per (head, q-chunk of 512): S^T tiles [128k, 512q] via
           64-contraction matmuls (two concurrent row-tiles T0/T8),
           exp on ACT from 2-bank PSUM groups -> bf16, O^T accumulation
           with V|ones (row 64 = softmax sums), per-q normalization via
           reciprocal + gpsimd partition broadcast.
  Phase C: out[qtile] = sum_h O_h^T.T @ Wo_h^T (fp32r), DMA to DRAM.
"""

import numpy as np

import concourse.bass as bass
import concourse.mybir as mybir
import concourse.tile as tile
from concourse import bacc
from concourse.masks import make_identity

F32 = mybir.dt.float32
F32R = mybir.dt.float32r
BF16 = mybir.dt.bfloat16

N_CORES = 8
B, S, D = 2, 4096, 768
H, DK = 12, 64
HPC = 3            # heads per core
QC = 512           # q-chunk width (free dim of S^T matmuls)
NQC = S // QC      # 8
NKB = S // 128     # 32 k-blocks of 128
XCH = 512          # x streaming chunk (columns of x^T per DMA)
OT_DT = F32R       # dtype of O^T staging

# projection group packing: 5 groups of two 64-dim tensors (by (head, kind))
# kind: 0=Q, 1=K, 2=V
PROJ_GROUPS = [((0, 0), (0, 1)), ((0, 2), (1, 0)), ((1, 1), (1, 2)),
               ((2, 0), (2, 1)), ((2, 2), (2, 2))]


def build_program(debug=False, repeat=1, mode="v2_e2"):
    nc = bacc.Bacc("TRN2", debug=False, num_devices=N_CORES)

    xT_d = nc.dram_tensor("xT", [D, S], F32R, kind="ExternalInput").ap()
    if mode.startswith("v2"):
        wp_d = nc.dram_tensor("wp", [HPC, 2, 6, 128, 128], F32R,
                              kind="ExternalInput").ap()
        bp_d = nc.dram_tensor("bp", [128, HPC, 2], F32,
                              kind="ExternalInput").ap()
    else:
        wp_d = nc.dram_tensor("wp", [5, 6, 128, 128], F32R,
                              kind="ExternalInput").ap()
        bp_d = nc.dram_tensor("bp", [128, 5], F32, kind="ExternalInput").ap()
    wo_d = nc.dram_tensor("wo", [HPC, DK, D], F32R, kind="ExternalInput").ap()
    out_d = nc.dram_tensor("out", [S, D], F32, kind="ExternalOutput").ap()

    dbg = {}
    if debug:
        dbg["qt"] = nc.dram_tensor("d_qt", [128, S], BF16,
                                   kind="ExternalOutput").ap()
        dbg["kt"] = nc.dram_tensor("d_kt", [128, S], BF16,
                                   kind="ExternalOutput").ap()
        dbg["v"] = nc.dram_tensor("d_v", [128, NKB, DK + 1], BF16,
                                  kind="ExternalOutput").ap()
        dbg["es"] = nc.dram_tensor("d_es", [128, 2, QC], BF16,
                                   kind="ExternalOutput").ap()
        dbg["po"] = nc.dram_tensor("d_po", [2, DK + 1, QC], F32,
                                   kind="ExternalOutput").ap()
        dbg["otr"] = nc.dram_tensor("d_otr", [DK + 1, QC], F32,
                                    kind="ExternalOutput").ap()
        dbg["rbc"] = nc.dram_tensor("d_rbc", [DK + 1, QC], F32,
                                    kind="ExternalOutput").ap()
        dbg["ot"] = nc.dram_tensor("d_ot", [DK + 1, S], F32,
                                   kind="ExternalOutput").ap()

    with tile.TileContext(nc) as tc, \
            nc.allow_low_precision("bf16/fp32r attention pipeline"):
        if mode.startswith("v2"):
            assert not debug and repeat >= 1
            for _ in range(repeat):
                _emit_v2(nc, tc, xT_d, wp_d, bp_d, wo_d, out_d,
                         exp_group=4 if mode == "v2_e4" else 2)
        else:
            _emit(nc, tc, xT_d, wp_d, bp_d, wo_d, out_d, dbg,
                  repeat=repeat, mode=mode)
    nc.compile()
    return nc


def _emit(nc, tc, xT_d, wp_d, bp_d, wo_d, out_d, dbg={},
          repeat=1, mode="tiled64"):
    import contextlib
    ctx = contextlib.ExitStack()
    with ctx:
        wpool = ctx.enter_context(tc.tile_pool(name="wpool", bufs=1))
        persist = ctx.enter_context(tc.tile_pool(name="persist", bufs=1))
        xpool = ctx.enter_context(tc.tile_pool(name="xpool", bufs=2))
        epool = ctx.enter_context(tc.tile_pool(name="epool", bufs=3))
        rpool = ctx.enter_context(tc.tile_pool(name="rpool", bufs=1))
        opool = ctx.enter_context(tc.tile_pool(name="opool", bufs=2))
        ppS = ctx.enter_context(tc.tile_pool(name="ppS", bufs=2, space="PSUM"))
        ppO = ctx.enter_context(tc.tile_pool(name="ppO", bufs=1, space="PSUM"))
        ppA = ctx.enter_context(tc.tile_pool(name="ppA", bufs=2, space="PSUM"))

        # ---- constants / weights ----
        wsb = wpool.tile([128, 5, 6, 128], F32R)
        nc.sync.dma_start(out=wsb, in_=wp_d.rearrange("g c p m -> p g c m"))
        bsb = wpool.tile([128, 5], F32)
        nc.sync.dma_start(out=bsb, in_=bp_d)
        wosb = wpool.tile([DK, HPC, D], F32R)
        nc.sync.dma_start(out=wosb, in_=wo_d.rearrange("j d m -> d j m"))
        ident = wpool.tile([128, 128], BF16)
        make_identity(nc, ident)

        assert not (dbg and repeat > 1)
        # which half each (head, kind) tensor is written to by the packed
        # projections, derived from PROJ_GROUPS
        wr_half = {}
        for gi, (mA, mB) in enumerate(PROJ_GROUPS):
            if gi == 4:
                wr_half[mA] = 0  # written to both halves
                continue
            wr_half[mA] = 0
            wr_half[mB] = 1

        for rep in range(repeat):
            # ---- persistent per-head tensors ----
            # QT/KT: [head_dim(64) in both halves (tiled64) or lower half +
            # zero upper (pad128), seq] bf16
            QT = [persist.tile([128, S], BF16, tag=f"qt{j}", name=f"qt{j}")
                  for j in range(HPC)]
            KT = [persist.tile([128, S], BF16, tag=f"kt{j}", name=f"kt{j}")
                  for j in range(HPC)]
            # V natural layout + ones column: [128 part = k%128, kb, 65]
            V = [persist.tile([128, NKB, DK + 1], BF16, tag=f"v{j}",
                              name=f"v{j}") for j in range(HPC)]
            # O^T staging: rows 0..63 = head dims, row 64 = softmax sums
            OT = [persist.tile([DK + 1, S], OT_DT, tag=f"ot{j}",
                               name=f"ot{j}") for j in range(HPC)]
            # VT transient [dims(64) at written half, seq] bf16
            VT = [persist.tile([128, S], BF16, tag=f"vt{j}", name=f"vt{j}")
                  for j in range(HPC)]

            for j in range(HPC):
                nc.vector.memset(V[j][:, :, DK], 1.0)

            def tgt(j, kind):
                return QT[j] if kind == 0 else KT[j] if kind == 1 else VT[j]

            # ---- Phase A: projections, x streamed in contraction-complete
            # column chunks ----
            n_xch = S // XCH
            for ci in range(n_xch):
                xq = xpool.tile([128, 6, XCH], F32R, tag="x", name="xq")
                nc.sync.dma_start(
                    out=xq,
                    in_=xT_d.rearrange("(c p) q -> p c q", p=128)[
                        :, :, ci * XCH:(ci + 1) * XCH],
                )
                for gi, (mA, mB) in enumerate(PROJ_GROUPS):
                    ps = ppA.tile([128, XCH], F32, tag="s", name="ps")
                    for c in range(6):
                        nc.tensor.matmul(
                            ps, lhsT=wsb[:, gi, c, :], rhs=xq[:, c, :],
                            start=(c == 0), stop=(c == 5))
                    # evacuate halves with bias add, cast to bf16
                    if gi == 4:
                        # V2 written to both halves at once (dup'd weights)
                        nc.vector.tensor_scalar_add(
                            out=VT[2][:, ci * XCH:(ci + 1) * XCH],
                            in0=ps, scalar1=bsb[:, gi:gi + 1])
                        continue
                    for half, (j, kind) in ((0, mA), (1, mB)):
                        lo, hi = half * 64, half * 64 + 64
                        nc.vector.tensor_scalar_add(
                            out=tgt(j, kind)[lo:hi, ci * XCH:(ci + 1) * XCH],
                            in0=ps[lo:hi, :],
                            scalar1=bsb[lo:hi, gi:gi + 1])

            # fix up Q/K halves (V^T needs none: transposes read the
            # written half directly)
            for j in range(HPC):
                for kind in (0, 1):
                    t = tgt(j, kind)
                    wh = wr_half[(j, kind)]
                    lo, hi = wh * 64, wh * 64 + 64
                    olo, ohi = 64 - lo, 128 - lo
                    if mode == "tiled64":
                        # duplicate into the other half
                        nc.sync.dma_start(out=t[olo:ohi, :], in_=t[lo:hi, :])
                    else:
                        # data to lower half, zero upper
                        if wh == 1:
                            nc.sync.dma_start(out=t[0:64, :], in_=t[64:128, :])
                        nc.vector.memset(t[64:128, :], 0.0)

            # V: transpose VT [dims, seq] -> natural [seq, dims] per block
            for j in range(HPC):
                voff = wr_half[(j, 2)] * 64
                for kb in range(NKB):
                    pt = ppA.tile([128, 128], BF16, tag="s", name="pt")
                    nc.tensor.transpose(
                        pt, VT[j][:, kb * 128:(kb + 1) * 128], ident)
                    nc.vector.tensor_copy(
                        out=V[j][:, kb, 0:DK], in_=pt[:, voff:voff + DK])

            if dbg:
                nc.sync.dma_start(out=dbg["qt"], in_=QT[0])
                nc.sync.dma_start(out=dbg["kt"], in_=KT[0])
                nc.sync.dma_start(out=dbg["v"], in_=V[0])

            # ---- Phase B: attention per head ----
            for j in range(HPC):
                for qi in range(NQC):
                    qs = qi * QC
                    poa = ppO.tile([DK + 1, QC], F32, tag="oa", name="poa")
                    if mode == "tiled64":
                        pob = ppO.tile([DK + 1, QC], F32, tag="ob",
                                       name="pob")
                    for p in range(NKB // 2):  # pairs of k-blocks
                        pss = ppS.tile([128, 2, QC], F32, tag="s", name="pss")
                        if mode == "tiled64":
                            # two concurrent 64-contraction row tiles
                            nc.tensor.matmul(
                                pss[:, 0, :],
                                lhsT=KT[j][0:64, p * 256:p * 256 + 128],
                                rhs=QT[j][0:64, qs:qs + QC],
                                start=True, stop=True)
                            nc.tensor.matmul(
                                pss[:, 1, :],
                                lhsT=KT[j][64:128, p * 256 + 128:p * 256 + 256],
                                rhs=QT[j][64:128, qs:qs + QC],
                                start=True, stop=True)
                        else:
                            for s in range(2):
                                kb = 2 * p + s
                                nc.tensor.matmul(
                                    pss[:, s, :],
                                    lhsT=KT[j][:, kb * 128:(kb + 1) * 128],
                                    rhs=QT[j][:, qs:qs + QC],
                                    start=True, stop=True)
                        es = epool.tile([128, 2, QC], BF16, tag="e", name="es")
                        nc.scalar.activation(
                            out=es, in_=pss,
                            func=mybir.ActivationFunctionType.Exp, scale=0.125)
                        if dbg and j == 0 and qi == 0 and p == 0:
                            nc.sync.dma_start(out=dbg["es"], in_=es)
                        for s in range(2):
                            kb = 2 * p + s
                            first = p == 0 and s == 0
                            last = p == NKB // 2 - 1 and s == 1
                            if mode == "tiled64":
                                nc.tensor.matmul(
                                    poa, lhsT=V[j][0:64, kb, :],
                                    rhs=es[0:64, s, :],
                                    start=first, stop=last,
                                    skip_group_check=True)
                                nc.tensor.matmul(
                                    pob, lhsT=V[j][64:128, kb, :],
                                    rhs=es[64:128, s, :],
                                    start=first, stop=last,
                                    skip_group_check=True)
                            else:
                                nc.tensor.matmul(
                                    poa, lhsT=V[j][:, kb, :],
                                    rhs=es[:, s, :],
                                    start=first, stop=last,
                                    skip_group_check=True)
                    # evacuate (DVE may read only one PSUM operand per op)
                    nc.vector.tensor_copy(out=OT[j][:, qs:qs + QC], in_=poa)
                    if mode == "tiled64":
                        nc.vector.tensor_add(
                            out=OT[j][:, qs:qs + QC],
                            in0=OT[j][:, qs:qs + QC], in1=pob)
                    # reciprocal of sums in place (row 64)
                    nc.vector.reciprocal(
                        out=OT[j][DK:DK + 1, qs:qs + QC],
                        in_=OT[j][DK:DK + 1, qs:qs + QC])
                    if dbg and j == 0 and qi == 0:
                        nc.sync.dma_start(
                            out=dbg["otr"],
                            in_=OT[0][:, 0:QC].bitcast(F32))
                    # broadcast recip across partitions and scale O^T.
                    # partition_broadcast reads PHYSICAL partition 0, so
                    # stage the recip row there via a tiny DMA first.
                    srow = rpool.tile([1, QC], OT_DT, tag="sr", name="srow")
                    nc.sync.dma_start(
                        out=srow, in_=OT[j][DK:DK + 1, qs:qs + QC])
                    rbc = rpool.tile([DK + 1, QC], OT_DT, tag="r", name="rbc")
                    nc.gpsimd.partition_broadcast(rbc, srow, channels=DK + 1)
                    if dbg and j == 0 and qi == 0:
                        nc.sync.dma_start(out=dbg["rbc"], in_=rbc.bitcast(F32))
                    nc.vector.tensor_mul(
                        out=OT[j][0:DK, qs:qs + QC],
                        in0=OT[j][0:DK, qs:qs + QC], in1=rbc[0:DK, :])

            if dbg:
                nc.sync.dma_start(out=dbg["ot"], in_=OT[0].bitcast(F32))

            # ---- Phase C: output projection ----
            for t in range(S // 128):
                c1 = ppA.tile([128, 512], F32, tag="s", name="c1")
                c2 = ppA.tile([128, 256], F32, tag="s", name="c2")
                for j in range(HPC):
                    nc.tensor.matmul(
                        c1, lhsT=OT[j][0:DK, t * 128:(t + 1) * 128],
                        rhs=wosb[:, j, 0:512],
                        start=(j == 0), stop=(j == HPC - 1))
                for j in range(HPC):
                    nc.tensor.matmul(
                        c2, lhsT=OT[j][0:DK, t * 128:(t + 1) * 128],
                        rhs=wosb[:, j, 512:768],
                        start=(j == 0), stop=(j == HPC - 1))
                ot = opool.tile([128, D], F32, tag="o", name="ot")
                nc.vector.tensor_copy(out=ot[:, 0:512], in_=c1)
                nc.vector.tensor_copy(out=ot[:, 512:768], in_=c2)
                nc.sync.dma_start(out=out_d[t * 128:(t + 1) * 128, :], in_=ot)




def _emit_v2(nc, tc, xT_d, wp_d, bp_d, wo_d, out_d, exp_group=4):
    """Per-head pipeline; S^T psum in bf16 when exp_group=4 (2048-wide exp)."""
    import contextlib
    ctx = contextlib.ExitStack()
    with ctx:
        wpool = ctx.enter_context(tc.tile_pool(name="wpool", bufs=1))
        persist = ctx.enter_context(tc.tile_pool(name="persist", bufs=1))
        hpool = ctx.enter_context(tc.tile_pool(name="hpool", bufs=2))
        xpool = ctx.enter_context(tc.tile_pool(name="xpool", bufs=2))
        epool = ctx.enter_context(tc.tile_pool(name="epool", bufs=4))
        rpool = ctx.enter_context(tc.tile_pool(name="rpool", bufs=1))
        opool = ctx.enter_context(tc.tile_pool(name="opool", bufs=2))
        # one shared PSUM pool for S-groups/proj/transposes/phase C
        # (3 slots of 2 banks) + the two O accumulators (1 bank each)
        ppS = ctx.enter_context(tc.tile_pool(name="ppS", bufs=3, space="PSUM"))
        ppO = ctx.enter_context(tc.tile_pool(name="ppO", bufs=1, space="PSUM"))
        ppA = ppS

        SDT = BF16 if exp_group == 4 else F32
        NG = NKB // exp_group

        wsb = wpool.tile([128, HPC, 2, 6, 128], F32R)
        nc.sync.dma_start(out=wsb, in_=wp_d.rearrange("j g c p m -> p j g c m"))
        bsb = wpool.tile([128, HPC, 2], F32)
        nc.sync.dma_start(out=bsb, in_=bp_d)
        wosb = wpool.tile([DK, HPC, D], F32R)
        nc.sync.dma_start(out=wosb, in_=wo_d.rearrange("j d m -> d j m"))
        ident = wpool.tile([128, 128], BF16)
        make_identity(nc, ident)

        OT = [persist.tile([DK + 1, S], OT_DT, tag=f"ot{j}", name=f"ot{j}")
              for j in range(HPC)]

        def emit_c(cqi):
            for t in range(cqi * QC // 128, (cqi + 1) * QC // 128):
                c1 = ppO.tile([128, 512], F32, tag="oa", name="c1")
                c2 = ppO.tile([128, 256], F32, tag="ob", name="c2")
                for jj in range(HPC):
                    nc.tensor.matmul(
                        c1, lhsT=OT[jj][0:DK, t * 128:(t + 1) * 128],
                        rhs=wosb[:, jj, 0:512],
                        start=(jj == 0), stop=(jj == HPC - 1))
                for jj in range(HPC):
                    nc.tensor.matmul(
                        c2, lhsT=OT[jj][0:DK, t * 128:(t + 1) * 128],
                        rhs=wosb[:, jj, 512:768],
                        start=(jj == 0), stop=(jj == HPC - 1))
                ot = opool.tile([128, D], F32, tag="o", name="ot")
                nc.vector.tensor_copy(out=ot[:, 0:512], in_=c1)
                nc.vector.tensor_copy(out=ot[:, 512:768], in_=c2)
                nc.sync.dma_start(
                    out=out_d[t * 128:(t + 1) * 128, :], in_=ot)

        n_xch = S // XCH
        for j in range(HPC):
            # ---- phase A for head j ----
            QT = hpool.tile([128, S], BF16, tag="qt", name="qt")
            KT = hpool.tile([128, S], BF16, tag="kt", name="kt")
            VT = hpool.tile([128, S], BF16, tag="vt", name="vt")
            V = hpool.tile([128, NKB, DK + 1], BF16, tag="v", name="v")
            nc.vector.memset(V[:, :, DK], 1.0)
            for ci in range(n_xch):
                xq = xpool.tile([128, 6, XCH], F32R, tag="x", name="xq")
                nc.sync.dma_start(
                    out=xq,
                    in_=xT_d.rearrange("(c p) q -> p c q", p=128)[
                        :, :, ci * XCH:(ci + 1) * XCH])
                cs = slice(ci * XCH, (ci + 1) * XCH)
                # group 0: (Q | K)
                ps = ppA.tile([128, XCH], F32, tag="s", name="ps")
                for c in range(6):
                    nc.tensor.matmul(
                        ps, lhsT=wsb[:, j, 0, c, :], rhs=xq[:, c, :],
                        start=(c == 0), stop=(c == 5))
                nc.vector.tensor_scalar_add(
                    out=QT[0:64, cs], in0=ps[0:64, :],
                    scalar1=bsb[0:64, j, 0:1])
                nc.vector.tensor_scalar_add(
                    out=KT[64:128, cs], in0=ps[64:128, :],
                    scalar1=bsb[64:128, j, 0:1])
                # group 1: (V | V) duplicated
                ps2 = ppA.tile([128, XCH], F32, tag="s", name="ps2")
                for c in range(6):
                    nc.tensor.matmul(
                        ps2, lhsT=wsb[:, j, 1, c, :], rhs=xq[:, c, :],
                        start=(c == 0), stop=(c == 5))
                nc.vector.tensor_scalar_add(
                    out=VT[:, cs], in0=ps2, scalar1=bsb[:, j, 1:2])
                # V natural layout via PE transposes (chunk's k-blocks)
                for kb in range(ci * XCH // 128, (ci + 1) * XCH // 128):
                    pt = ppA.tile([128, 128], BF16, tag="s", name="pt")
                    nc.tensor.transpose(
                        pt, VT[:, kb * 128:(kb + 1) * 128], ident)
                    nc.vector.tensor_copy(
                        out=V[:, kb, 0:DK], in_=pt[:, 0:DK])
            # duplicate halves: Q lower->upper, K upper->lower
            nc.sync.dma_start(out=QT[64:128, :], in_=QT[0:64, :])
            nc.sync.dma_start(out=KT[0:64, :], in_=KT[64:128, :])

            # ---- phase B for head j ----
            for qi in range(NQC):
                qs = qi * QC
                poa = ppO.tile([DK + 1, QC], F32, tag="oa", name="poa")
                pob = ppO.tile([DK + 1, QC], F32, tag="ob", name="pob")
                for g in range(NG):
                    pss = ppS.tile([128, exp_group, QC], SDT, tag="s",
                                   name="pss")
                    # T0 row-tile: first half of the group's k-blocks;
                    # T8: second half (separate PSUM banks)
                    hg = exp_group // 2
                    for i in range(hg):
                        kb = g * exp_group + i
                        nc.tensor.matmul(
                            pss[:, i, :],
                            lhsT=KT[0:64, kb * 128:(kb + 1) * 128],
                            rhs=QT[0:64, qs:qs + QC],
                            start=True, stop=True)
                    for i in range(hg):
                        kb = g * exp_group + hg + i
                        nc.tensor.matmul(
                            pss[:, hg + i, :],
                            lhsT=KT[64:128, kb * 128:(kb + 1) * 128],
                            rhs=QT[64:128, qs:qs + QC],
                            start=True, stop=True)
                    es = epool.tile([128, exp_group, QC], BF16, tag="e",
                                    name="es")
                    nc.scalar.activation(
                        out=es, in_=pss,
                        func=mybir.ActivationFunctionType.Exp, scale=0.125)
                    for s in range(exp_group):
                        kb = g * exp_group + s
                        first = g == 0 and s == 0
                        last = g == NG - 1 and s == exp_group - 1
                        nc.tensor.matmul(
                            poa, lhsT=V[0:64, kb, :], rhs=es[0:64, s, :],
                            start=first, stop=last, skip_group_check=True)
                        nc.tensor.matmul(
                            pob, lhsT=V[64:128, kb, :], rhs=es[64:128, s, :],
                            start=first, stop=last, skip_group_check=True)
                nc.vector.tensor_copy(out=OT[j][:, qs:qs + QC], in_=poa)
                nc.vector.tensor_add(
                    out=OT[j][:, qs:qs + QC],
                    in0=OT[j][:, qs:qs + QC], in1=pob)
                nc.vector.reciprocal(
                    out=OT[j][DK:DK + 1, qs:qs + QC],
                    in_=OT[j][DK:DK + 1, qs:qs + QC])
                srow = rpool.tile([1, QC], OT_DT, tag="sr", name="srow")
                nc.sync.dma_start(
                    out=srow, in_=OT[j][DK:DK + 1, qs:qs + QC])
                rbc = rpool.tile([DK + 1, QC], OT_DT, tag="r", name="rbc")
                nc.gpsimd.partition_broadcast(rbc, srow, channels=DK + 1)
                nc.vector.tensor_mul(
                    out=OT[j][0:DK, qs:qs + QC],
                    in0=OT[j][0:DK, qs:qs + QC], in1=rbc[0:DK, :])

        # ---- phase C: output projection (borrows psumO slots) ----
        for cqi in range(NQC):
            emit_c(cqi)




# ---------------------------------------------------------------------------
# host side
# ---------------------------------------------------------------------------

KERNEL_MODE = "v2_e2"


def shard_inputs(x, Wq, bq, Wk, bk, Wv, bv, Wo, bo, mode=None):
    """Build the 8 per-core input maps."""
    mode = mode or KERNEL_MODE
    if mode.startswith("v2"):
        return shard_inputs_v2(x, Wq, bq, Wk, bk, Wv, bv, Wo, bo)
    return shard_inputs_v1(x, Wq, bq, Wk, bk, Wv, bv, Wo, bo)


def shard_inputs_v2(x, Wq, bq, Wk, bk, Wv, bv, Wo, bo):
    x = np.asarray(x, np.float32)
    Wq, Wk, Wv = (np.asarray(a, np.float32) for a in (Wq, Wk, Wv))
    bq, bk, bv = (np.asarray(a, np.float32) for a in (bq, bk, bv))
    Wo = np.asarray(Wo, np.float32)
    in_maps = []
    for c in range(N_CORES):
        b, g = divmod(c, 4)
        heads = [3 * g + j for j in range(HPC)]
        wp = np.empty((HPC, 2, 6, 128, 128), np.float32)
        bp = np.zeros((128, HPC, 2), np.float32)
        wo = np.empty((HPC, DK, D), np.float32)
        for j, h in enumerate(heads):
            sl = slice(64 * h, 64 * h + 64)
            wp[j, 0, :, :, 0:64] = Wq[sl].T.reshape(6, 128, 64)
            wp[j, 0, :, :, 64:128] = Wk[sl].T.reshape(6, 128, 64)
            wp[j, 1, :, :, 0:64] = Wv[sl].T.reshape(6, 128, 64)
            wp[j, 1, :, :, 64:128] = Wv[sl].T.reshape(6, 128, 64)
            bp[0:64, j, 0] = bq[sl]
            bp[64:128, j, 0] = bk[sl]
            bp[0:64, j, 1] = bv[sl]
            bp[64:128, j, 1] = bv[sl]
            wo[j] = Wo[:, sl].T
        in_maps.append({
            "xT": np.ascontiguousarray(x[b].T),
            "wp": wp, "bp": bp, "wo": wo,
        })
    return in_maps


def shard_inputs_v1(x, Wq, bq, Wk, bk, Wv, bv, Wo, bo):
    """Build the 8 per-core input maps."""
    x = np.asarray(x, np.float32)
    Ws = {0: np.asarray(Wq, np.float32), 1: np.asarray(Wk, np.float32),
          2: np.asarray(Wv, np.float32)}
    bs = {0: np.asarray(bq, np.float32), 1: np.asarray(bk, np.float32),
          2: np.asarray(bv, np.float32)}
    Wo = np.asarray(Wo, np.float32)
    in_maps = []
    for c in range(N_CORES):
        b, g = divmod(c, 4)
        heads = [3 * g + j for j in range(HPC)]
        wp = np.empty((5, 6, 128, 128), np.float32)
        bp = np.zeros((128, 5), np.float32)
        for gi, (mA, mB) in enumerate(PROJ_GROUPS):
            for half, (j, kind) in ((0, mA), (1, mB)):
                h = heads[j]
                Wh = Ws[kind][64 * h:64 * h + 64, :]       # [64, 768]
                chunks = Wh.T.reshape(6, 128, 64)          # [c, p, 64]
                wp[gi, :, :, half * 64:half * 64 + 64] = chunks
                bp[half * 64:half * 64 + 64, gi] = bs[kind][64 * h:64 * h + 64]
        wo = np.empty((HPC, DK, D), np.float32)
        for j in range(HPC):
            h = heads[j]
            wo[j] = Wo[:, 64 * h:64 * h + 64].T
        in_maps.append({
            "xT": np.ascontiguousarray(x[b].T),
            "wp": wp, "bp": bp, "wo": wo,
        })
    return in_maps


def assemble_output(parts, bo):
    out = np.empty((B, S, D), np.float32)
    for b in range(B):
        acc = parts[4 * b]["out"].astype(np.float32).copy()
        for c in range(4 * b + 1, 4 * b + 4):
            acc += parts[c]["out"]
        out[b] = acc + np.asarray(bo, np.float32)[None, :]
    return out


_RUNNER = None


def _make_runner(nc):
    """Reusable PJRT runner (mirrors bass2jax.run_bass_via_pjrt multi-core)."""
    import jax
    import jax.numpy as jnp
    from jax.experimental.shard_map import shard_map
    from jax.sharding import Mesh, PartitionSpec
    from concourse import bass2jax

    bass2jax.install_neuronx_cc_hook()

    partition_name = (nc.partition_id_tensor.name
                      if nc.partition_id_tensor else None)
    in_names, out_names, out_avals = [], [], []
    for alloc in nc.m.functions[0].allocations:
        if not isinstance(alloc, mybir.MemoryLocationSet):
            continue
        name = alloc.memorylocations[0].name
        if alloc.kind == "ExternalInput":
            if name != partition_name:
                in_names.append(name)
        elif alloc.kind == "ExternalOutput":
            out_names.append(name)
            out_avals.append(jax.core.ShapedArray(
                tuple(alloc.tensor_shape), mybir.dt.np(alloc.dtype)))
    n_params = len(in_names)
    n_outs = len(out_names)
    all_in_names = list(in_names) + list(out_names)
    if partition_name is not None:
        all_in_names.append(partition_name)
    donate = tuple(range(n_params, n_params + n_outs))

    def _body(*args):
        operands = list(args)
        if partition_name is not None:
            operands.append(bass2jax.partition_id_tensor())
        outs = bass2jax._bass_exec_p.bind(
            *operands,
            out_avals=tuple(out_avals),
            in_names=tuple(all_in_names),
            out_names=tuple(out_names),
            lowering_input_output_aliases=(),
            sim_require_finite=True,
            sim_require_nnan=True,
            nc=nc,
        )
        return tuple(outs)

    devices = jax.devices()[:N_CORES]
    mesh = Mesh(np.asarray(devices), ("core",))
    in_specs = (PartitionSpec("core"),) * (n_params + n_outs)
    out_specs = (PartitionSpec("core"),) * n_outs
    sharded = jax.jit(
        shard_map(_body, mesh=mesh, in_specs=in_specs, out_specs=out_specs,
                  check_rep=False),
        donate_argnums=donate, keep_unused=True)

    def run(in_maps):
        per_core = [[np.asarray(m[name]) for name in in_names]
                    for m in in_maps]
        concat_in = [np.concatenate([per_core[c][i] for c in range(N_CORES)],
                                    axis=0) for i in range(n_params)]
        zeros = [np.zeros((N_CORES * av.shape[0], *av.shape[1:]), av.dtype)
                 for av in out_avals]
        outs = sharded(*concat_in, *zeros)
        return [
            {name: np.asarray(outs[i]).reshape(N_CORES, *out_avals[i].shape)[c]
             for i, name in enumerate(out_names)}
            for c in range(N_CORES)
        ]

    run.sharded = sharded
    run.in_names = in_names
    run.out_names = out_names
    run.out_avals = out_avals
    run.n_params = n_params
    return run


def get_runner():
    global _RUNNER
    if _RUNNER is None:
        nc = build_program()
        _RUNNER = _make_runner(nc)
    return _RUNNER


def kernel(x, Wq, bq, Wk, bk, Wv, bv, Wo, bo):
    run = get_runner()
    in_maps = shard_inputs(x, Wq, bq, Wk, bk, Wv, bv, Wo, bo)
    parts = run(in_maps)
    return assemble_output(parts, bo)



# revision 6
# speedup vs baseline: 1.1044x; 1.1044x over previous
"""Multi-head attention (B=2, S=4096, D=768, H=12, d_k=64) on 8 TRN2 cores.

Sharding: core c -> batch b = c//4, head group g = c%4 (heads 3g..3g+2).
Each core computes partial = sum_{h in group} softmax(QK^T/8) V @ Wo_h^T
over its batch; host sums the 4 partials per batch and adds bo.

Device kernel (identical SPMD program, per-core data):
  Phase A: QKV projections (fp32r matmuls), Q^T/K^T/V^T produced in
           [head_dim, seq] layout (bf16), V transposed to natural
           [seq, head_dim] layout with a ones column appended (row sums).
  Phase B: per (head, q-chunk of 512): S^T tiles [128k, 512q] via
           64-contraction matmuls (two concurrent row-tiles T0/T8),
           exp on ACT from 2-bank PSUM groups -> bf16, O^T accumulation
           with V|ones (row 64 = softmax sums), per-q normalization via
           reciprocal + gpsimd partition broadcast.
  Phase C: out[qtile] = sum_h O_h^T.T @ Wo_h^T (fp32r), DMA to DRAM.
"""

import numpy as np

import concourse.bass as bass
import concourse.mybir as mybir
import concourse.tile as tile
from concourse import bacc
from concourse.masks import make_identity

F32 = mybir.dt.float32
F32R = mybir.dt.float32r
BF16 = mybir.dt.bfloat16

N_CORES = 8
B, S, D = 2, 4096, 768
H, DK = 12, 64
HPC = 3            # heads per core
QC = 512           # q-chunk width (free dim of S^T matmuls)
NQC = S // QC      # 8
NKB = S // 128     # 32 k-blocks of 128
XCH = 512          # x streaming chunk (columns of x^T per DMA)
OT_DT = F32R       # dtype of O^T staging

# projection group packing: 5 groups of two 64-dim tensors (by (head, kind))
# kind: 0=Q, 1=K, 2=V
PROJ_GROUPS = [((0, 0), (0, 1)), ((0, 2), (1, 0)), ((1, 1), (1, 2)),
               ((2, 0), (2, 1)), ((2, 2), (2, 2))]


def build_program(debug=False, repeat=1, mode=None):
    mode = mode or KERNEL_MODE
    nc = bacc.Bacc("TRN2", debug=False, num_devices=N_CORES)

    xT_d = nc.dram_tensor("xT", [D, S], F32R, kind="ExternalInput").ap()
    if mode.startswith("v2") or mode.startswith("v3"):
        wp_d = nc.dram_tensor("wp", [HPC, 2, 6, 128, 128], F32R,
                              kind="ExternalInput").ap()
        bp_d = nc.dram_tensor("bp", [128, HPC, 2], F32,
                              kind="ExternalInput").ap()
    else:
        wp_d = nc.dram_tensor("wp", [5, 6, 128, 128], F32R,
                              kind="ExternalInput").ap()
        bp_d = nc.dram_tensor("bp", [128, 5], F32, kind="ExternalInput").ap()
    wo_d = nc.dram_tensor("wo", [HPC, DK, D], F32R, kind="ExternalInput").ap()
    out_d = nc.dram_tensor("out", [S, D], F32, kind="ExternalOutput").ap()

    dbg = {}
    if debug:
        dbg["qt"] = nc.dram_tensor("d_qt", [128, S], BF16,
                                   kind="ExternalOutput").ap()
        dbg["kt"] = nc.dram_tensor("d_kt", [128, S], BF16,
                                   kind="ExternalOutput").ap()
        dbg["v"] = nc.dram_tensor("d_v", [128, NKB, DK + 1], BF16,
                                  kind="ExternalOutput").ap()
        dbg["es"] = nc.dram_tensor("d_es", [128, 2, QC], BF16,
                                   kind="ExternalOutput").ap()
        dbg["po"] = nc.dram_tensor("d_po", [2, DK + 1, QC], F32,
                                   kind="ExternalOutput").ap()
        dbg["otr"] = nc.dram_tensor("d_otr", [DK + 1, QC], F32,
                                    kind="ExternalOutput").ap()
        dbg["rbc"] = nc.dram_tensor("d_rbc", [DK + 1, QC], F32,
                                    kind="ExternalOutput").ap()
        dbg["ot"] = nc.dram_tensor("d_ot", [DK + 1, S], F32,
                                   kind="ExternalOutput").ap()

    with tile.TileContext(nc) as tc, \
            nc.allow_low_precision("bf16/fp32r attention pipeline"):
        if mode.startswith("v3"):
            assert not debug and repeat >= 1
            for _ in range(repeat):
                _emit_v3(nc, tc, xT_d, wp_d, bp_d, wo_d, out_d)
        elif mode.startswith("v2"):
            assert not debug and repeat >= 1
            for _ in range(repeat):
                _emit_v2(nc, tc, xT_d, wp_d, bp_d, wo_d, out_d,
                         exp_group=4 if mode == "v2_e4" else 2)
        else:
            _emit(nc, tc, xT_d, wp_d, bp_d, wo_d, out_d, dbg,
                  repeat=repeat, mode=mode)
    nc.compile()
    return nc


def _emit(nc, tc, xT_d, wp_d, bp_d, wo_d, out_d, dbg={},
          repeat=1, mode="tiled64"):
    import contextlib
    ctx = contextlib.ExitStack()
    with ctx:
        wpool = ctx.enter_context(tc.tile_pool(name="wpool", bufs=1))
        persist = ctx.enter_context(tc.tile_pool(name="persist", bufs=1))
        xpool = ctx.enter_context(tc.tile_pool(name="xpool", bufs=2))
        epool = ctx.enter_context(tc.tile_pool(name="epool", bufs=3))
        rpool = ctx.enter_context(tc.tile_pool(name="rpool", bufs=1))
        opool = ctx.enter_context(tc.tile_pool(name="opool", bufs=2))
        ppS = ctx.enter_context(tc.tile_pool(name="ppS", bufs=2, space="PSUM"))
        ppO = ctx.enter_context(tc.tile_pool(name="ppO", bufs=1, space="PSUM"))
        ppA = ctx.enter_context(tc.tile_pool(name="ppA", bufs=2, space="PSUM"))

        # ---- constants / weights ----
        wsb = wpool.tile([128, 5, 6, 128], F32R)
        nc.sync.dma_start(out=wsb, in_=wp_d.rearrange("g c p m -> p g c m"))
        bsb = wpool.tile([128, 5], F32)
        nc.sync.dma_start(out=bsb, in_=bp_d)
        wosb = wpool.tile([DK, HPC, D], F32R)
        nc.sync.dma_start(out=wosb, in_=wo_d.rearrange("j d m -> d j m"))
        ident = wpool.tile([128, 128], BF16)
        make_identity(nc, ident)

        assert not (dbg and repeat > 1)
        # which half each (head, kind) tensor is written to by the packed
        # projections, derived from PROJ_GROUPS
        wr_half = {}
        for gi, (mA, mB) in enumerate(PROJ_GROUPS):
            if gi == 4:
                wr_half[mA] = 0  # written to both halves
                continue
            wr_half[mA] = 0
            wr_half[mB] = 1

        for rep in range(repeat):
            # ---- persistent per-head tensors ----
            # QT/KT: [head_dim(64) in both halves (tiled64) or lower half +
            # zero upper (pad128), seq] bf16
            QT = [persist.tile([128, S], BF16, tag=f"qt{j}", name=f"qt{j}")
                  for j in range(HPC)]
            KT = [persist.tile([128, S], BF16, tag=f"kt{j}", name=f"kt{j}")
                  for j in range(HPC)]
            # V natural layout + ones column: [128 part = k%128, kb, 65]
            V = [persist.tile([128, NKB, DK + 1], BF16, tag=f"v{j}",
                              name=f"v{j}") for j in range(HPC)]
            # O^T staging: rows 0..63 = head dims, row 64 = softmax sums
            OT = [persist.tile([DK + 1, S], OT_DT, tag=f"ot{j}",
                               name=f"ot{j}") for j in range(HPC)]
            # VT transient [dims(64) at written half, seq] bf16
            VT = [persist.tile([128, S], BF16, tag=f"vt{j}", name=f"vt{j}")
                  for j in range(HPC)]

            for j in range(HPC):
                nc.vector.memset(V[j][:, :, DK], 1.0)

            def tgt(j, kind):
                return QT[j] if kind == 0 else KT[j] if kind == 1 else VT[j]

            # ---- Phase A: projections, x streamed in contraction-complete
            # column chunks ----
            n_xch = S // XCH
            for ci in range(n_xch):
                xq = xpool.tile([128, 6, XCH], F32R, tag="x", name="xq")
                nc.sync.dma_start(
                    out=xq,
                    in_=xT_d.rearrange("(c p) q -> p c q", p=128)[
                        :, :, ci * XCH:(ci + 1) * XCH],
                )
                for gi, (mA, mB) in enumerate(PROJ_GROUPS):
                    ps = ppA.tile([128, XCH], F32, tag="s", name="ps")
                    for c in range(6):
                        nc.tensor.matmul(
                            ps, lhsT=wsb[:, gi, c, :], rhs=xq[:, c, :],
                            start=(c == 0), stop=(c == 5))
                    # evacuate halves with bias add, cast to bf16
                    if gi == 4:
                        # V2 written to both halves at once (dup'd weights)
                        nc.vector.tensor_scalar_add(
                            out=VT[2][:, ci * XCH:(ci + 1) * XCH],
                            in0=ps, scalar1=bsb[:, gi:gi + 1])
                        continue
                    for half, (j, kind) in ((0, mA), (1, mB)):
                        lo, hi = half * 64, half * 64 + 64
                        nc.vector.tensor_scalar_add(
                            out=tgt(j, kind)[lo:hi, ci * XCH:(ci + 1) * XCH],
                            in0=ps[lo:hi, :],
                            scalar1=bsb[lo:hi, gi:gi + 1])

            # fix up Q/K halves (V^T needs none: transposes read the
            # written half directly)
            for j in range(HPC):
                for kind in (0, 1):
                    t = tgt(j, kind)
                    wh = wr_half[(j, kind)]
                    lo, hi = wh * 64, wh * 64 + 64
                    olo, ohi = 64 - lo, 128 - lo
                    if mode == "tiled64":
                        # duplicate into the other half
                        nc.sync.dma_start(out=t[olo:ohi, :], in_=t[lo:hi, :])
                    else:
                        # data to lower half, zero upper
                        if wh == 1:
                            nc.sync.dma_start(out=t[0:64, :], in_=t[64:128, :])
                        nc.vector.memset(t[64:128, :], 0.0)

            # V: transpose VT [dims, seq] -> natural [seq, dims] per block
            for j in range(HPC):
                voff = wr_half[(j, 2)] * 64
                for kb in range(NKB):
                    pt = ppA.tile([128, 128], BF16, tag="s", name="pt")
                    nc.tensor.transpose(
                        pt, VT[j][:, kb * 128:(kb + 1) * 128], ident)
                    nc.vector.tensor_copy(
                        out=V[j][:, kb, 0:DK], in_=pt[:, voff:voff + DK])

            if dbg:
                nc.sync.dma_start(out=dbg["qt"], in_=QT[0])
                nc.sync.dma_start(out=dbg["kt"], in_=KT[0])
                nc.sync.dma_start(out=dbg["v"], in_=V[0])

            # ---- Phase B: attention per head ----
            for j in range(HPC):
                for qi in range(NQC):
                    qs = qi * QC
                    poa = ppO.tile([DK + 1, QC], F32, tag="oa", name="poa")
                    if mode == "tiled64":
                        pob = ppO.tile([DK + 1, QC], F32, tag="ob",
                                       name="pob")
                    for p in range(NKB // 2):  # pairs of k-blocks
                        pss = ppS.tile([128, 2, QC], F32, tag="s", name="pss")
                        if mode == "tiled64":
                            # two concurrent 64-contraction row tiles
                            nc.tensor.matmul(
                                pss[:, 0, :],
                                lhsT=KT[j][0:64, p * 256:p * 256 + 128],
                                rhs=QT[j][0:64, qs:qs + QC],
                                start=True, stop=True)
                            nc.tensor.matmul(
                                pss[:, 1, :],
                                lhsT=KT[j][64:128, p * 256 + 128:p * 256 + 256],
                                rhs=QT[j][64:128, qs:qs + QC],
                                start=True, stop=True)
                        else:
                            for s in range(2):
                                kb = 2 * p + s
                                nc.tensor.matmul(
                                    pss[:, s, :],
                                    lhsT=KT[j][:, kb * 128:(kb + 1) * 128],
                                    rhs=QT[j][:, qs:qs + QC],
                                    start=True, stop=True)
                        es = epool.tile([128, 2, QC], BF16, tag="e", name="es")
                        nc.scalar.activation(
                            out=es, in_=pss,
                            func=mybir.ActivationFunctionType.Exp, scale=0.125)
                        if dbg and j == 0 and qi == 0 and p == 0:
                            nc.sync.dma_start(out=dbg["es"], in_=es)
                        for s in range(2):
                            kb = 2 * p + s
                            first = p == 0 and s == 0
                            last = p == NKB // 2 - 1 and s == 1
                            if mode == "tiled64":
                                nc.tensor.matmul(
                                    poa, lhsT=V[j][0:64, kb, :],
                                    rhs=es[0:64, s, :],
                                    start=first, stop=last,
                                    skip_group_check=True)
                                nc.tensor.matmul(
                                    pob, lhsT=V[j][64:128, kb, :],
                                    rhs=es[64:128, s, :],
                                    start=first, stop=last,
                                    skip_group_check=True)
                            else:
                                nc.tensor.matmul(
                                    poa, lhsT=V[j][:, kb, :],
                                    rhs=es[:, s, :],
                                    start=first, stop=last,
                                    skip_group_check=True)
                    # evacuate (DVE may read only one PSUM operand per op)
                    nc.vector.tensor_copy(out=OT[j][:, qs:qs + QC], in_=poa)
                    if mode == "tiled64":
                        nc.vector.tensor_add(
                            out=OT[j][:, qs:qs + QC],
                            in0=OT[j][:, qs:qs + QC], in1=pob)
                    # reciprocal of sums in place (row 64)
                    nc.vector.reciprocal(
                        out=OT[j][DK:DK + 1, qs:qs + QC],
                        in_=OT[j][DK:DK + 1, qs:qs + QC])
                    if dbg and j == 0 and qi == 0:
                        nc.sync.dma_start(
                            out=dbg["otr"],
                            in_=OT[0][:, 0:QC].bitcast(F32))
                    # broadcast recip across partitions and scale O^T.
                    # partition_broadcast reads PHYSICAL partition 0, so
                    # stage the recip row there via a tiny DMA first.
                    srow = rpool.tile([1, QC], OT_DT, tag="sr", name="srow")
                    nc.sync.dma_start(
                        out=srow, in_=OT[j][DK:DK + 1, qs:qs + QC])
                    rbc = rpool.tile([DK + 1, QC], OT_DT, tag="r", name="rbc")
                    nc.gpsimd.partition_broadcast(rbc, srow, channels=DK + 1)
                    if dbg and j == 0 and qi == 0:
                        nc.sync.dma_start(out=dbg["rbc"], in_=rbc.bitcast(F32))
                    nc.vector.tensor_mul(
                        out=OT[j][0:DK, qs:qs + QC],
                        in0=OT[j][0:DK, qs:qs + QC], in1=rbc[0:DK, :])

            if dbg:
                nc.sync.dma_start(out=dbg["ot"], in_=OT[0].bitcast(F32))

            # ---- Phase C: output projection ----
            for t in range(S // 128):
                c1 = ppA.tile([128, 512], F32, tag="s", name="c1")
                c2 = ppA.tile([128, 256], F32, tag="s", name="c2")
                for j in range(HPC):
                    nc.tensor.matmul(
                        c1, lhsT=OT[j][0:DK, t * 128:(t + 1) * 128],
                        rhs=wosb[:, j, 0:512],
                        start=(j == 0), stop=(j == HPC - 1))
                for j in range(HPC):
                    nc.tensor.matmul(
                        c2, lhsT=OT[j][0:DK, t * 128:(t + 1) * 128],
                        rhs=wosb[:, j, 512:768],
                        start=(j == 0), stop=(j == HPC - 1))
                ot = opool.tile([128, D], F32, tag="o", name="ot")
                nc.vector.tensor_copy(out=ot[:, 0:512], in_=c1)
                nc.vector.tensor_copy(out=ot[:, 512:768], in_=c2)
                nc.sync.dma_start(out=out_d[t * 128:(t + 1) * 128, :], in_=ot)




def _emit_v2(nc, tc, xT_d, wp_d, bp_d, wo_d, out_d, exp_group=4):
    """Per-head pipeline; S^T psum in bf16 when exp_group=4 (2048-wide exp)."""
    import contextlib
    ctx = contextlib.ExitStack()
    with ctx:
        wpool = ctx.enter_context(tc.tile_pool(name="wpool", bufs=1))
        persist = ctx.enter_context(tc.tile_pool(name="persist", bufs=1))
        hpool = ctx.enter_context(tc.tile_pool(name="hpool", bufs=2))
        xpool = ctx.enter_context(tc.tile_pool(name="xpool", bufs=2))
        epool = ctx.enter_context(tc.tile_pool(name="epool", bufs=4))
        rpool = ctx.enter_context(tc.tile_pool(name="rpool", bufs=1))
        opool = ctx.enter_context(tc.tile_pool(name="opool", bufs=2))
        # one shared PSUM pool for S-groups/proj/transposes/phase C
        # (3 slots of 2 banks) + the two O accumulators (1 bank each)
        ppS = ctx.enter_context(tc.tile_pool(name="ppS", bufs=3, space="PSUM"))
        ppO = ctx.enter_context(tc.tile_pool(name="ppO", bufs=1, space="PSUM"))
        ppA = ppS

        SDT = BF16 if exp_group == 4 else F32
        NG = NKB // exp_group

        wsb = wpool.tile([128, HPC, 2, 6, 128], F32R)
        nc.sync.dma_start(out=wsb, in_=wp_d.rearrange("j g c p m -> p j g c m"))
        bsb = wpool.tile([128, HPC, 2], F32)
        nc.sync.dma_start(out=bsb, in_=bp_d)
        wosb = wpool.tile([DK, HPC, D], F32R)
        nc.sync.dma_start(out=wosb, in_=wo_d.rearrange("j d m -> d j m"))
        ident = wpool.tile([128, 128], BF16)
        make_identity(nc, ident)

        OT = [persist.tile([DK + 1, S], OT_DT, tag=f"ot{j}", name=f"ot{j}")
              for j in range(HPC)]

        def emit_c(cqi):
            for t in range(cqi * QC // 128, (cqi + 1) * QC // 128):
                c1 = ppO.tile([128, 512], F32, tag="oa", name="c1")
                c2 = ppO.tile([128, 256], F32, tag="ob", name="c2")
                for jj in range(HPC):
                    nc.tensor.matmul(
                        c1, lhsT=OT[jj][0:DK, t * 128:(t + 1) * 128],
                        rhs=wosb[:, jj, 0:512],
                        start=(jj == 0), stop=(jj == HPC - 1))
                for jj in range(HPC):
                    nc.tensor.matmul(
                        c2, lhsT=OT[jj][0:DK, t * 128:(t + 1) * 128],
                        rhs=wosb[:, jj, 512:768],
                        start=(jj == 0), stop=(jj == HPC - 1))
                ot = opool.tile([128, D], F32, tag="o", name="ot")
                nc.vector.tensor_copy(out=ot[:, 0:512], in_=c1)
                nc.vector.tensor_copy(out=ot[:, 512:768], in_=c2)
                nc.sync.dma_start(
                    out=out_d[t * 128:(t + 1) * 128, :], in_=ot)

        n_xch = S // XCH
        for j in range(HPC):
            # ---- phase A for head j ----
            QT = hpool.tile([128, S], BF16, tag="qt", name="qt")
            KT = hpool.tile([128, S], BF16, tag="kt", name="kt")
            VT = hpool.tile([128, S], BF16, tag="vt", name="vt")
            V = hpool.tile([128, NKB, DK + 1], BF16, tag="v", name="v")
            nc.vector.memset(V[:, :, DK], 1.0)
            for ci in range(n_xch):
                xq = xpool.tile([128, 6, XCH], F32R, tag="x", name="xq")
                nc.sync.dma_start(
                    out=xq,
                    in_=xT_d.rearrange("(c p) q -> p c q", p=128)[
                        :, :, ci * XCH:(ci + 1) * XCH])
                cs = slice(ci * XCH, (ci + 1) * XCH)
                # group 0: (Q | K)
                ps = ppA.tile([128, XCH], F32, tag="s", name="ps")
                for c in range(6):
                    nc.tensor.matmul(
                        ps, lhsT=wsb[:, j, 0, c, :], rhs=xq[:, c, :],
                        start=(c == 0), stop=(c == 5))
                nc.vector.tensor_scalar_add(
                    out=QT[0:64, cs], in0=ps[0:64, :],
                    scalar1=bsb[0:64, j, 0:1])
                nc.vector.tensor_scalar_add(
                    out=KT[64:128, cs], in0=ps[64:128, :],
                    scalar1=bsb[64:128, j, 0:1])
                # group 1: (V | V) duplicated
                ps2 = ppA.tile([128, XCH], F32, tag="s", name="ps2")
                for c in range(6):
                    nc.tensor.matmul(
                        ps2, lhsT=wsb[:, j, 1, c, :], rhs=xq[:, c, :],
                        start=(c == 0), stop=(c == 5))
                nc.vector.tensor_scalar_add(
                    out=VT[:, cs], in0=ps2, scalar1=bsb[:, j, 1:2])
                # V natural layout via PE transposes (chunk's k-blocks)
                for kb in range(ci * XCH // 128, (ci + 1) * XCH // 128):
                    pt = ppA.tile([128, 128], BF16, tag="s", name="pt")
                    nc.tensor.transpose(
                        pt, VT[:, kb * 128:(kb + 1) * 128], ident)
                    nc.vector.tensor_copy(
                        out=V[:, kb, 0:DK], in_=pt[:, 0:DK])
            # duplicate halves: Q lower->upper, K upper->lower
            nc.sync.dma_start(out=QT[64:128, :], in_=QT[0:64, :])
            nc.sync.dma_start(out=KT[0:64, :], in_=KT[64:128, :])

            # ---- phase B for head j ----
            for qi in range(NQC):
                qs = qi * QC
                poa = ppO.tile([DK + 1, QC], F32, tag="oa", name="poa")
                pob = ppO.tile([DK + 1, QC], F32, tag="ob", name="pob")
                for g in range(NG):
                    pss = ppS.tile([128, exp_group, QC], SDT, tag="s",
                                   name="pss")
                    # T0 row-tile: first half of the group's k-blocks;
                    # T8: second half (separate PSUM banks)
                    hg = exp_group // 2
                    for i in range(hg):
                        kb = g * exp_group + i
                        nc.tensor.matmul(
                            pss[:, i, :],
                            lhsT=KT[0:64, kb * 128:(kb + 1) * 128],
                            rhs=QT[0:64, qs:qs + QC],
                            start=True, stop=True)
                    for i in range(hg):
                        kb = g * exp_group + hg + i
                        nc.tensor.matmul(
                            pss[:, hg + i, :],
                            lhsT=KT[64:128, kb * 128:(kb + 1) * 128],
                            rhs=QT[64:128, qs:qs + QC],
                            start=True, stop=True)
                    es = epool.tile([128, exp_group, QC], BF16, tag="e",
                                    name="es")
                    nc.scalar.activation(
                        out=es, in_=pss,
                        func=mybir.ActivationFunctionType.Exp, scale=0.125)
                    for s in range(exp_group):
                        kb = g * exp_group + s
                        first = g == 0 and s == 0
                        last = g == NG - 1 and s == exp_group - 1
                        nc.tensor.matmul(
                            poa, lhsT=V[0:64, kb, :], rhs=es[0:64, s, :],
                            start=first, stop=last, skip_group_check=True)
                        nc.tensor.matmul(
                            pob, lhsT=V[64:128, kb, :], rhs=es[64:128, s, :],
                            start=first, stop=last, skip_group_check=True)
                nc.vector.tensor_copy(out=OT[j][:, qs:qs + QC], in_=poa)
                nc.vector.tensor_add(
                    out=OT[j][:, qs:qs + QC],
                    in0=OT[j][:, qs:qs + QC], in1=pob)
                nc.vector.reciprocal(
                    out=OT[j][DK:DK + 1, qs:qs + QC],
                    in_=OT[j][DK:DK + 1, qs:qs + QC])
                srow = rpool.tile([1, QC], OT_DT, tag="sr", name="srow")
                nc.sync.dma_start(
                    out=srow, in_=OT[j][DK:DK + 1, qs:qs + QC])
                rbc = rpool.tile([DK + 1, QC], OT_DT, tag="r", name="rbc")
                nc.gpsimd.partition_broadcast(rbc, srow, channels=DK + 1)
                nc.vector.tensor_mul(
                    out=OT[j][0:DK, qs:qs + QC],
                    in0=OT[j][0:DK, qs:qs + QC], in1=rbc[0:DK, :])

        # ---- phase C: output projection (borrows psumO slots) ----
        for cqi in range(NQC):
            emit_c(cqi)




def _emit_v3(nc, tc, xT_d, wp_d, bp_d, wo_d, out_d):
    """v3: full-contraction O matmuls (single accumulator), separate PSUM
    pools per phase, software-pipelined exp->O, and phase A(j+1)/C emission
    interleaved into phase B(j) q-chunks."""
    import contextlib
    ctx = contextlib.ExitStack()
    with ctx:
        wpool = ctx.enter_context(tc.tile_pool(name="wpool", bufs=1))
        otpool = ctx.enter_context(tc.tile_pool(name="otpool", bufs=1))
        hpool = ctx.enter_context(tc.tile_pool(name="hpool", bufs=2))
        xpool = ctx.enter_context(tc.tile_pool(name="xpool", bufs=2))
        epool = ctx.enter_context(tc.tile_pool(name="epool", bufs=4))
        rpool = ctx.enter_context(tc.tile_pool(name="rpool", bufs=1))
        opool = ctx.enter_context(tc.tile_pool(name="opool", bufs=2))
        # PSUM budget (8 banks): S-score slots 2x2, O-accum/phase-C 2x1,
        # phase-A proj/transpose 2x1
        ppS = ctx.enter_context(tc.tile_pool(name="ppS", bufs=2, space="PSUM"))
        ppO = ctx.enter_context(tc.tile_pool(name="ppO", bufs=2, space="PSUM"))
        ppA = ctx.enter_context(tc.tile_pool(name="ppA", bufs=2, space="PSUM"))

        wsb = wpool.tile([128, HPC, 2, 6, 128], F32R)
        nc.sync.dma_start(out=wsb, in_=wp_d.rearrange("j g c p m -> p j g c m"))
        bsb = wpool.tile([128, HPC, 2], F32)
        nc.sync.dma_start(out=bsb, in_=bp_d)
        wosb = wpool.tile([DK, HPC, D], F32R)
        nc.sync.dma_start(out=wosb, in_=wo_d.rearrange("j d m -> d j m"))
        ident = wpool.tile([128, 128], BF16)
        make_identity(nc, ident)

        OT = [otpool.tile([DK + 1, S], OT_DT, tag=f"ot{j}", name=f"ot{j}")
              for j in range(HPC)]

        n_xch = S // XCH

        def alloc_head(j):
            t = {
                "QT": hpool.tile([128, S], BF16, tag="qt", name="qt"),
                "KT": hpool.tile([128, S], BF16, tag="kt", name="kt"),
                "VT": hpool.tile([128, S], BF16, tag="vt", name="vt"),
                "V": hpool.tile([128, NKB, DK + 1], BF16, tag="v", name="v"),
            }
            nc.vector.memset(t["V"][:, :, DK], 1.0)
            return t

        def emit_A(j, t, ci):
            QT, KT, VT, V = t["QT"], t["KT"], t["VT"], t["V"]
            xq = xpool.tile([128, 6, XCH], F32R, tag="x", name="xq")
            nc.sync.dma_start(
                out=xq,
                in_=xT_d.rearrange("(c p) q -> p c q", p=128)[
                    :, :, ci * XCH:(ci + 1) * XCH])
            cs = slice(ci * XCH, (ci + 1) * XCH)
            # group 0: (Q | K)
            ps = ppA.tile([128, XCH], F32, tag="a", name="ps")
            for c in range(6):
                nc.tensor.matmul(
                    ps, lhsT=wsb[:, j, 0, c, :], rhs=xq[:, c, :],
                    start=(c == 0), stop=(c == 5))
            nc.vector.tensor_scalar_add(
                out=QT[0:64, cs], in0=ps[0:64, :], scalar1=bsb[0:64, j, 0:1])
            nc.vector.tensor_scalar_add(
                out=KT[64:128, cs], in0=ps[64:128, :],
                scalar1=bsb[64:128, j, 0:1])
            # group 1: (V | V) duplicated
            ps2 = ppA.tile([128, XCH], F32, tag="a", name="ps2")
            for c in range(6):
                nc.tensor.matmul(
                    ps2, lhsT=wsb[:, j, 1, c, :], rhs=xq[:, c, :],
                    start=(c == 0), stop=(c == 5))
            nc.vector.tensor_scalar_add(
                out=VT[:, cs], in0=ps2, scalar1=bsb[:, j, 1:2])
            # V natural layout via PE transposes
            for kb in range(ci * XCH // 128, (ci + 1) * XCH // 128):
                pt = ppA.tile([128, 128], BF16, tag="a", name="pt")
                nc.tensor.transpose(
                    pt, VT[:, kb * 128:(kb + 1) * 128], ident)
                nc.vector.tensor_copy(out=V[:, kb, 0:DK], in_=pt[:, 0:DK])

        def finish_A(t):
            nc.sync.dma_start(out=t["QT"][64:128, :], in_=t["QT"][0:64, :])
            nc.sync.dma_start(out=t["KT"][0:64, :], in_=t["KT"][64:128, :])

        def emit_B_chunk(j, t, qi):
            QT, KT, V = t["QT"], t["KT"], t["V"]
            qs = qi * QC
            poa = ppO.tile([DK + 1, QC], F32, tag="oa", name="poa")

            def emit_O(es, g):
                for s in range(2):
                    kb = 2 * g + s
                    nc.tensor.matmul(
                        poa, lhsT=V[:, kb, :], rhs=es[:, s, :],
                        start=(kb == 0), stop=(kb == NKB - 1),
                        skip_group_check=True)

            pend = None
            for g in range(NKB // 2):
                pss = ppS.tile([128, 2, QC], F32, tag="s", name="pss")
                nc.tensor.matmul(
                    pss[:, 0, :],
                    lhsT=KT[0:64, (2 * g) * 128:(2 * g + 1) * 128],
                    rhs=QT[0:64, qs:qs + QC], start=True, stop=True)
                nc.tensor.matmul(
                    pss[:, 1, :],
                    lhsT=KT[64:128, (2 * g + 1) * 128:(2 * g + 2) * 128],
                    rhs=QT[64:128, qs:qs + QC], start=True, stop=True)
                es = epool.tile([128, 2, QC], BF16, tag="e", name="es")
                nc.scalar.activation(
                    out=es, in_=pss,
                    func=mybir.ActivationFunctionType.Exp, scale=0.125)
                if pend is not None:
                    emit_O(*pend)
                pend = (es, g)
            emit_O(*pend)

            nc.vector.tensor_copy(out=OT[j][:, qs:qs + QC], in_=poa)
            nc.vector.reciprocal(
                out=OT[j][DK:DK + 1, qs:qs + QC],
                in_=OT[j][DK:DK + 1, qs:qs + QC])
            srow = rpool.tile([1, QC], OT_DT, tag="sr", name="srow")
            nc.sync.dma_start(out=srow, in_=OT[j][DK:DK + 1, qs:qs + QC])
            rbc = rpool.tile([DK + 1, QC], OT_DT, tag="r", name="rbc")
            nc.gpsimd.partition_broadcast(rbc, srow, channels=DK + 1)
            nc.vector.tensor_mul(
                out=OT[j][0:DK, qs:qs + QC],
                in0=OT[j][0:DK, qs:qs + QC], in1=rbc[0:DK, :])

        def emit_C_range(qi):
            for ti in range(qi * QC // 128, (qi + 1) * QC // 128):
                c1 = ppO.tile([128, 512], F32, tag="oa", name="c1")
                for jj in range(HPC):
                    nc.tensor.matmul(
                        c1, lhsT=OT[jj][0:DK, ti * 128:(ti + 1) * 128],
                        rhs=wosb[:, jj, 0:512],
                        start=(jj == 0), stop=(jj == HPC - 1))
                c2 = ppO.tile([128, 256], F32, tag="oa", name="c2")
                for jj in range(HPC):
                    nc.tensor.matmul(
                        c2, lhsT=OT[jj][0:DK, ti * 128:(ti + 1) * 128],
                        rhs=wosb[:, jj, 512:768],
                        start=(jj == 0), stop=(jj == HPC - 1))
                ot = opool.tile([128, D], F32, tag="o", name="ot")
                nc.vector.tensor_copy(out=ot[:, 0:512], in_=c1)
                nc.vector.tensor_copy(out=ot[:, 512:768], in_=c2)
                nc.sync.dma_start(
                    out=out_d[ti * 128:(ti + 1) * 128, :], in_=ot)

        # prologue: head 0 phase A
        heads = [alloc_head(0)]
        for ci in range(n_xch):
            emit_A(0, heads[0], ci)
        finish_A(heads[0])
        for j in range(HPC):
            t = heads[j]
            if j + 1 < HPC:
                heads.append(alloc_head(j + 1))
            for qi in range(NQC):
                emit_B_chunk(j, t, qi)
                if j + 1 < HPC:
                    emit_A(j + 1, heads[j + 1], qi)
                else:
                    emit_C_range(qi)
            if j + 1 < HPC:
                finish_A(heads[j + 1])


# ---------------------------------------------------------------------------
# host side
# ---------------------------------------------------------------------------

KERNEL_MODE = "v3"


def shard_inputs(x, Wq, bq, Wk, bk, Wv, bv, Wo, bo, mode=None):
    """Build the 8 per-core input maps."""
    mode = mode or KERNEL_MODE
    if mode.startswith("v2") or mode.startswith("v3"):
        return shard_inputs_v2(x, Wq, bq, Wk, bk, Wv, bv, Wo, bo)
    return shard_inputs_v1(x, Wq, bq, Wk, bk, Wv, bv, Wo, bo)


def shard_inputs_v2(x, Wq, bq, Wk, bk, Wv, bv, Wo, bo):
    x = np.asarray(x, np.float32)
    Wq, Wk, Wv = (np.asarray(a, np.float32) for a in (Wq, Wk, Wv))
    bq, bk, bv = (np.asarray(a, np.float32) for a in (bq, bk, bv))
    Wo = np.asarray(Wo, np.float32)
    in_maps = []
    for c in range(N_CORES):
        b, g = divmod(c, 4)
        heads = [3 * g + j for j in range(HPC)]
        wp = np.empty((HPC, 2, 6, 128, 128), np.float32)
        bp = np.zeros((128, HPC, 2), np.float32)
        wo = np.empty((HPC, DK, D), np.float32)
        for j, h in enumerate(heads):
            sl = slice(64 * h, 64 * h + 64)
            wp[j, 0, :, :, 0:64] = Wq[sl].T.reshape(6, 128, 64)
            wp[j, 0, :, :, 64:128] = Wk[sl].T.reshape(6, 128, 64)
            wp[j, 1, :, :, 0:64] = Wv[sl].T.reshape(6, 128, 64)
            wp[j, 1, :, :, 64:128] = Wv[sl].T.reshape(6, 128, 64)
            bp[0:64, j, 0] = bq[sl]
            bp[64:128, j, 0] = bk[sl]
            bp[0:64, j, 1] = bv[sl]
            bp[64:128, j, 1] = bv[sl]
            wo[j] = Wo[:, sl].T
        in_maps.append({
            "xT": np.ascontiguousarray(x[b].T),
            "wp": wp, "bp": bp, "wo": wo,
        })
    return in_maps


def shard_inputs_v1(x, Wq, bq, Wk, bk, Wv, bv, Wo, bo):
    """Build the 8 per-core input maps."""
    x = np.asarray(x, np.float32)
    Ws = {0: np.asarray(Wq, np.float32), 1: np.asarray(Wk, np.float32),
          2: np.asarray(Wv, np.float32)}
    bs = {0: np.asarray(bq, np.float32), 1: np.asarray(bk, np.float32),
          2: np.asarray(bv, np.float32)}
    Wo = np.asarray(Wo, np.float32)
    in_maps = []
    for c in range(N_CORES):
        b, g = divmod(c, 4)
        heads = [3 * g + j for j in range(HPC)]
        wp = np.empty((5, 6, 128, 128), np.float32)
        bp = np.zeros((128, 5), np.float32)
        for gi, (mA, mB) in enumerate(PROJ_GROUPS):
            for half, (j, kind) in ((0, mA), (1, mB)):
                h = heads[j]
                Wh = Ws[kind][64 * h:64 * h + 64, :]       # [64, 768]
                chunks = Wh.T.reshape(6, 128, 64)          # [c, p, 64]
                wp[gi, :, :, half * 64:half * 64 + 64] = chunks
                bp[half * 64:half * 64 + 64, gi] = bs[kind][64 * h:64 * h + 64]
        wo = np.empty((HPC, DK, D), np.float32)
        for j in range(HPC):
            h = heads[j]
            wo[j] = Wo[:, 64 * h:64 * h + 64].T
        in_maps.append({
            "xT": np.ascontiguousarray(x[b].T),
            "wp": wp, "bp": bp, "wo": wo,
        })
    return in_maps


def assemble_output(parts, bo):
    out = np.empty((B, S, D), np.float32)
    for b in range(B):
        acc = parts[4 * b]["out"].astype(np.float32).copy()
        for c in range(4 * b + 1, 4 * b + 4):
            acc += parts[c]["out"]
        out[b] = acc + np.asarray(bo, np.float32)[None, :]
    return out


_RUNNER = None


def _make_runner(nc):
    """Reusable PJRT runner (mirrors bass2jax.run_bass_via_pjrt multi-core)."""
    import jax
    import jax.numpy as jnp
    from jax.experimental.shard_map import shard_map
    from jax.sharding import Mesh, PartitionSpec
    from concourse import bass2jax

    bass2jax.install_neuronx_cc_hook()

    partition_name = (nc.partition_id_tensor.name
                      if nc.partition_id_tensor else None)
    in_names, out_names, out_avals = [], [], []
    for alloc in nc.m.functions[0].allocations:
        if not isinstance(alloc, mybir.MemoryLocationSet):
            continue
        name = alloc.memorylocations[0].name
        if alloc.kind == "ExternalInput":
            if name != partition_name:
                in_names.append(name)
        elif alloc.kind == "ExternalOutput":
            out_names.append(name)
            out_avals.append(jax.core.ShapedArray(
                tuple(alloc.tensor_shape), mybir.dt.np(alloc.dtype)))
    n_params = len(in_names)
    n_outs = len(out_names)
    all_in_names = list(in_names) + list(out_names)
    if partition_name is not None:
        all_in_names.append(partition_name)
    donate = tuple(range(n_params, n_params + n_outs))

    def _body(*args):
        operands = list(args)
        if partition_name is not None:
            operands.append(bass2jax.partition_id_tensor())
        outs = bass2jax._bass_exec_p.bind(
            *operands,
            out_avals=tuple(out_avals),
            in_names=tuple(all_in_names),
            out_names=tuple(out_names),
            lowering_input_output_aliases=(),
            sim_require_finite=True,
            sim_require_nnan=True,
            nc=nc,
        )
        return tuple(outs)

    devices = jax.devices()[:N_CORES]
    mesh = Mesh(np.asarray(devices), ("core",))
    in_specs = (PartitionSpec("core"),) * (n_params + n_outs)
    out_specs = (PartitionSpec("core"),) * n_outs
    sharded = jax.jit(
        shard_map(_body, mesh=mesh, in_specs=in_specs, out_specs=out_specs,
                  check_rep=False),
        donate_argnums=donate, keep_unused=True)

    def run(in_maps):
        per_core = [[np.asarray(m[name]) for name in in_names]
                    for m in in_maps]
        concat_in = [np.concatenate([per_core[c][i] for c in range(N_CORES)],
                                    axis=0) for i in range(n_params)]
        zeros = [np.zeros((N_CORES * av.shape[0], *av.shape[1:]), av.dtype)
                 for av in out_avals]
        outs = sharded(*concat_in, *zeros)
        return [
            {name: np.asarray(outs[i]).reshape(N_CORES, *out_avals[i].shape)[c]
             for i, name in enumerate(out_names)}
            for c in range(N_CORES)
        ]

    run.sharded = sharded
    run.in_names = in_names
    run.out_names = out_names
    run.out_avals = out_avals
    run.n_params = n_params
    return run


def get_runner():
    global _RUNNER
    if _RUNNER is None:
        nc = build_program()
        _RUNNER = _make_runner(nc)
    return _RUNNER


def kernel(x, Wq, bq, Wk, bk, Wv, bv, Wo, bo):
    run = get_runner()
    in_maps = shard_inputs(x, Wq, bq, Wk, bk, Wv, bv, Wo, bo)
    parts = run(in_maps)
    return assemble_output(parts, bo)



# revision 56
# speedup vs baseline: 1.2934x; 1.1711x over previous
"""Multi-head attention (B=2, S=4096, D=768, H=12, d_k=64) on 8 TRN2 cores.

Sharding: core c -> batch b = c//4, head group g = c%4 (heads 3g..3g+2).
Each core computes partial = sum_{h in group} softmax(QK^T/8) V @ Wo_h^T
over its batch; host sums the 4 partials per batch and adds bo.

Device kernel (identical SPMD program, per-core data):
  Phase A: QKV projections (fp32r matmuls), Q^T/K^T/V^T produced in
           [head_dim, seq] layout (bf16), V transposed to natural
           [seq, head_dim] layout with a ones column appended (row sums).
  Phase B: per (head, q-chunk of 512): S^T tiles [128k, 512q] via
           64-contraction matmuls (two concurrent row-tiles T0/T8),
           exp on ACT from 2-bank PSUM groups -> bf16, O^T accumulation
           with V|ones (row 64 = softmax sums), per-q normalization via
           reciprocal + gpsimd partition broadcast.
  Phase C: out[qtile] = sum_h O_h^T.T @ Wo_h^T (fp32r), DMA to DRAM.
"""

import numpy as np

import concourse.bass as bass
import concourse.mybir as mybir
import concourse.tile as tile
from concourse import bacc
from concourse.masks import make_identity

F32 = mybir.dt.float32
FP16 = mybir.dt.float16
I16 = mybir.dt.int16
F32R = mybir.dt.float32r
BF16 = mybir.dt.bfloat16

N_CORES = 8
B, S, D = 2, 4096, 768
H, DK = 12, 64
HPC = 3            # heads per core
QC = 512           # q-chunk width (free dim of S^T matmuls)
NQC = S // QC      # 8
NKB = S // 128     # 32 k-blocks of 128
XCH = 512          # x streaming chunk (columns of x^T per DMA)
OT_DT = F32R       # dtype of O^T staging

# projection group packing: 5 groups of two 64-dim tensors (by (head, kind))
# kind: 0=Q, 1=K, 2=V
PROJ_GROUPS = [((0, 0), (0, 1)), ((0, 2), (1, 0)), ((1, 1), (1, 2)),
               ((2, 0), (2, 1)), ((2, 2), (2, 2))]


def build_program(debug=False, repeat=1, mode=None):
    mode = mode or KERNEL_MODE
    nc = bacc.Bacc("TRN2", debug=False, num_devices=N_CORES)

    xT_d = nc.dram_tensor("xT", [D, S], F32R, kind="ExternalInput").ap()
    if mode.startswith(("v2", "v3", "v4", "v5")):
        wp_d = nc.dram_tensor("wp", [HPC, 2, 6, 128, 128], F32R,
                              kind="ExternalInput").ap()
        bp_d = nc.dram_tensor("bp", [128, HPC, 2], F32,
                              kind="ExternalInput").ap()
    else:
        wp_d = nc.dram_tensor("wp", [5, 6, 128, 128], F32R,
                              kind="ExternalInput").ap()
        bp_d = nc.dram_tensor("bp", [128, 5], F32, kind="ExternalInput").ap()
    wo_d = nc.dram_tensor("wo", [HPC, DK, D], F32R, kind="ExternalInput").ap()
    out_d = nc.dram_tensor("out", [S, D], F32, kind="ExternalOutput").ap()

    dbg = {}
    if debug:
        dbg["qt"] = nc.dram_tensor("d_qt", [128, S], BF16,
                                   kind="ExternalOutput").ap()
        dbg["kt"] = nc.dram_tensor("d_kt", [128, S], BF16,
                                   kind="ExternalOutput").ap()
        dbg["v"] = nc.dram_tensor("d_v", [128, NKB, DK + 1], BF16,
                                  kind="ExternalOutput").ap()
        dbg["es"] = nc.dram_tensor("d_es", [128, 2, QC], BF16,
                                   kind="ExternalOutput").ap()
        dbg["po"] = nc.dram_tensor("d_po", [2, DK + 1, QC], F32,
                                   kind="ExternalOutput").ap()
        dbg["otr"] = nc.dram_tensor("d_otr", [DK + 1, QC], F32,
                                    kind="ExternalOutput").ap()
        dbg["rbc"] = nc.dram_tensor("d_rbc", [DK + 1, QC], F32,
                                    kind="ExternalOutput").ap()
        dbg["ot"] = nc.dram_tensor("d_ot", [DK + 1, S], F32,
                                   kind="ExternalOutput").ap()

    with tile.TileContext(nc) as tc, \
            nc.allow_low_precision("bf16/fp32r attention pipeline"):
        if mode.startswith("mb"):
            for _ in range(repeat):
                _emit_mb(nc, tc, out_d, paired=(mode == "mba"))
        elif mode.startswith(("v3", "v4", "v5")):
            assert not debug and repeat >= 1
            for _ in range(repeat):
                _emit_v3(nc, tc, xT_d, wp_d, bp_d, wo_d, out_d,
                         no_c=("xC" in mode), no_o=("xO" in mode),
                         no_exp=("xE" in mode),
                         packed=("p" in mode or mode.startswith("v5")),
                         early=("e" in mode or mode.startswith("v5")),
                         split_o=mode.startswith("v5"),
                         ot_bf16=("b" in mode),
                         dve_groups=DVE_PATTERNS.get(mode, frozenset()))
        elif mode.startswith("v2"):
            assert not debug and repeat >= 1
            for _ in range(repeat):
                _emit_v2(nc, tc, xT_d, wp_d, bp_d, wo_d, out_d,
                         exp_group=4 if mode == "v2_e4" else 2)
        else:
            _emit(nc, tc, xT_d, wp_d, bp_d, wo_d, out_d, dbg,
                  repeat=repeat, mode=mode)
    nc.compile()
    return nc


def _emit(nc, tc, xT_d, wp_d, bp_d, wo_d, out_d, dbg={},
          repeat=1, mode="tiled64"):
    import contextlib
    ctx = contextlib.ExitStack()
    with ctx:
        wpool = ctx.enter_context(tc.tile_pool(name="wpool", bufs=1))
        persist = ctx.enter_context(tc.tile_pool(name="persist", bufs=1))
        xpool = ctx.enter_context(tc.tile_pool(name="xpool", bufs=2))
        epool = ctx.enter_context(tc.tile_pool(name="epool", bufs=3))
        rpool = ctx.enter_context(tc.tile_pool(name="rpool", bufs=1))
        opool = ctx.enter_context(tc.tile_pool(name="opool", bufs=2))
        ppS = ctx.enter_context(tc.tile_pool(name="ppS", bufs=2, space="PSUM"))
        ppO = ctx.enter_context(tc.tile_pool(name="ppO", bufs=1, space="PSUM"))
        ppA = ctx.enter_context(tc.tile_pool(name="ppA", bufs=2, space="PSUM"))

        # ---- constants / weights ----
        wsb = wpool.tile([128, 5, 6, 128], F32R)
        nc.sync.dma_start(out=wsb, in_=wp_d.rearrange("g c p m -> p g c m"))
        bsb = wpool.tile([128, 5], F32)
        nc.sync.dma_start(out=bsb, in_=bp_d)
        wosb = wpool.tile([DK, HPC, D], F32R)
        nc.sync.dma_start(out=wosb, in_=wo_d.rearrange("j d m -> d j m"))
        ident = wpool.tile([128, 128], BF16)
        make_identity(nc, ident)

        assert not (dbg and repeat > 1)
        # which half each (head, kind) tensor is written to by the packed
        # projections, derived from PROJ_GROUPS
        wr_half = {}
        for gi, (mA, mB) in enumerate(PROJ_GROUPS):
            if gi == 4:
                wr_half[mA] = 0  # written to both halves
                continue
            wr_half[mA] = 0
            wr_half[mB] = 1

        for rep in range(repeat):
            # ---- persistent per-head tensors ----
            # QT/KT: [head_dim(64) in both halves (tiled64) or lower half +
            # zero upper (pad128), seq] bf16
            QT = [persist.tile([128, S], BF16, tag=f"qt{j}", name=f"qt{j}")
                  for j in range(HPC)]
            KT = [persist.tile([128, S], BF16, tag=f"kt{j}", name=f"kt{j}")
                  for j in range(HPC)]
            # V natural layout + ones column: [128 part = k%128, kb, 65]
            V = [persist.tile([128, NKB, DK + 1], BF16, tag=f"v{j}",
                              name=f"v{j}") for j in range(HPC)]
            # O^T staging: rows 0..63 = head dims, row 64 = softmax sums
            OT = [persist.tile([DK + 1, S], OT_DT, tag=f"ot{j}",
                               name=f"ot{j}") for j in range(HPC)]
            # VT transient [dims(64) at written half, seq] bf16
            VT = [persist.tile([128, S], BF16, tag=f"vt{j}", name=f"vt{j}")
                  for j in range(HPC)]

            for j in range(HPC):
                nc.vector.memset(V[j][:, :, DK], 1.0)

            def tgt(j, kind):
                return QT[j] if kind == 0 else KT[j] if kind == 1 else VT[j]

            # ---- Phase A: projections, x streamed in contraction-complete
            # column chunks ----
            n_xch = S // XCH
            for ci in range(n_xch):
                xq = xpool.tile([128, 6, XCH], F32R, tag="x", name="xq")
                nc.sync.dma_start(
                    out=xq,
                    in_=xT_d.rearrange("(c p) q -> p c q", p=128)[
                        :, :, ci * XCH:(ci + 1) * XCH],
                )
                for gi, (mA, mB) in enumerate(PROJ_GROUPS):
                    ps = ppA.tile([128, XCH], F32, tag="s", name="ps")
                    for c in range(6):
                        nc.tensor.matmul(
                            ps, lhsT=wsb[:, gi, c, :], rhs=xq[:, c, :],
                            start=(c == 0), stop=(c == 5))
                    # evacuate halves with bias add, cast to bf16
                    if gi == 4:
                        # V2 written to both halves at once (dup'd weights)
                        nc.vector.tensor_scalar_add(
                            out=VT[2][:, ci * XCH:(ci + 1) * XCH],
                            in0=ps, scalar1=bsb[:, gi:gi + 1])
                        continue
                    for half, (j, kind) in ((0, mA), (1, mB)):
                        lo, hi = half * 64, half * 64 + 64
                        nc.vector.tensor_scalar_add(
                            out=tgt(j, kind)[lo:hi, ci * XCH:(ci + 1) * XCH],
                            in0=ps[lo:hi, :],
                            scalar1=bsb[lo:hi, gi:gi + 1])

            # fix up Q/K halves (V^T needs none: transposes read the
            # written half directly)
            for j in range(HPC):
                for kind in (0, 1):
                    t = tgt(j, kind)
                    wh = wr_half[(j, kind)]
                    lo, hi = wh * 64, wh * 64 + 64
                    olo, ohi = 64 - lo, 128 - lo
                    if mode == "tiled64":
                        # duplicate into the other half
                        nc.sync.dma_start(out=t[olo:ohi, :], in_=t[lo:hi, :])
                    else:
                        # data to lower half, zero upper
                        if wh == 1:
                            nc.sync.dma_start(out=t[0:64, :], in_=t[64:128, :])
                        nc.vector.memset(t[64:128, :], 0.0)

            # V: transpose VT [dims, seq] -> natural [seq, dims] per block
            for j in range(HPC):
                voff = wr_half[(j, 2)] * 64
                for kb in range(NKB):
                    pt = ppA.tile([128, 128], BF16, tag="s", name="pt")
                    nc.tensor.transpose(
                        pt, VT[j][:, kb * 128:(kb + 1) * 128], ident)
                    nc.vector.tensor_copy(
                        out=V[j][:, kb, 0:DK], in_=pt[:, voff:voff + DK])

            if dbg:
                nc.sync.dma_start(out=dbg["qt"], in_=QT[0])
                nc.sync.dma_start(out=dbg["kt"], in_=KT[0])
                nc.sync.dma_start(out=dbg["v"], in_=V[0])

            # ---- Phase B: attention per head ----
            for j in range(HPC):
                for qi in range(NQC):
                    qs = qi * QC
                    poa = ppO.tile([DK + 1, QC], F32, tag="oa", name="poa")
                    if mode == "tiled64":
                        pob = ppO.tile([DK + 1, QC], F32, tag="ob",
                                       name="pob")
                    for p in range(NKB // 2):  # pairs of k-blocks
                        pss = ppS.tile([128, 2, QC], F32, tag="s", name="pss")
                        if mode == "tiled64":
                            # two concurrent 64-contraction row tiles
                            nc.tensor.matmul(
                                pss[:, 0, :],
                                lhsT=KT[j][0:64, p * 256:p * 256 + 128],
                                rhs=QT[j][0:64, qs:qs + QC],
                                start=True, stop=True)
                            nc.tensor.matmul(
                                pss[:, 1, :],
                                lhsT=KT[j][64:128, p * 256 + 128:p * 256 + 256],
                                rhs=QT[j][64:128, qs:qs + QC],
                                start=True, stop=True)
                        else:
                            for s in range(2):
                                kb = 2 * p + s
                                nc.tensor.matmul(
                                    pss[:, s, :],
                                    lhsT=KT[j][:, kb * 128:(kb + 1) * 128],
                                    rhs=QT[j][:, qs:qs + QC],
                                    start=True, stop=True)
                        es = epool.tile([128, 2, QC], BF16, tag="e", name="es")
                        nc.scalar.activation(
                            out=es, in_=pss,
                            func=mybir.ActivationFunctionType.Exp, scale=0.125)
                        if dbg and j == 0 and qi == 0 and p == 0:
                            nc.sync.dma_start(out=dbg["es"], in_=es)
                        for s in range(2):
                            kb = 2 * p + s
                            first = p == 0 and s == 0
                            last = p == NKB // 2 - 1 and s == 1
                            if mode == "tiled64":
                                nc.tensor.matmul(
                                    poa, lhsT=V[j][0:64, kb, :],
                                    rhs=es[0:64, s, :],
                                    start=first, stop=last,
                                    skip_group_check=True)
                                nc.tensor.matmul(
                                    pob, lhsT=V[j][64:128, kb, :],
                                    rhs=es[64:128, s, :],
                                    start=first, stop=last,
                                    skip_group_check=True)
                            else:
                                nc.tensor.matmul(
                                    poa, lhsT=V[j][:, kb, :],
                                    rhs=es[:, s, :],
                                    start=first, stop=last,
                                    skip_group_check=True)
                    # evacuate (DVE may read only one PSUM operand per op)
                    nc.vector.tensor_copy(out=OT[j][:, qs:qs + QC], in_=poa)
                    if mode == "tiled64":
                        nc.vector.tensor_add(
                            out=OT[j][:, qs:qs + QC],
                            in0=OT[j][:, qs:qs + QC], in1=pob)
                    # reciprocal of sums in place (row 64)
                    nc.vector.reciprocal(
                        out=OT[j][DK:DK + 1, qs:qs + QC],
                        in_=OT[j][DK:DK + 1, qs:qs + QC])
                    if dbg and j == 0 and qi == 0:
                        nc.sync.dma_start(
                            out=dbg["otr"],
                            in_=OT[0][:, 0:QC].bitcast(F32))
                    # broadcast recip across partitions and scale O^T.
                    # partition_broadcast reads PHYSICAL partition 0, so
                    # stage the recip row there via a tiny DMA first.
                    srow = rpool.tile([1, QC], OT_DT, tag="sr", name="srow")
                    nc.sync.dma_start(
                        out=srow, in_=OT[j][DK:DK + 1, qs:qs + QC])
                    rbc = rpool.tile([DK + 1, QC], OT_DT, tag="r", name="rbc")
                    nc.gpsimd.partition_broadcast(rbc, srow, channels=DK + 1)
                    if dbg and j == 0 and qi == 0:
                        nc.sync.dma_start(out=dbg["rbc"], in_=rbc.bitcast(F32))
                    nc.vector.tensor_mul(
                        out=OT[j][0:DK, qs:qs + QC],
                        in0=OT[j][0:DK, qs:qs + QC], in1=rbc[0:DK, :])

            if dbg:
                nc.sync.dma_start(out=dbg["ot"], in_=OT[0].bitcast(F32))

            # ---- Phase C: output projection ----
            for t in range(S // 128):
                c1 = ppA.tile([128, 512], F32, tag="s", name="c1")
                c2 = ppA.tile([128, 256], F32, tag="s", name="c2")
                for j in range(HPC):
                    nc.tensor.matmul(
                        c1, lhsT=OT[j][0:DK, t * 128:(t + 1) * 128],
                        rhs=wosb[:, j, 0:512],
                        start=(j == 0), stop=(j == HPC - 1))
                for j in range(HPC):
                    nc.tensor.matmul(
                        c2, lhsT=OT[j][0:DK, t * 128:(t + 1) * 128],
                        rhs=wosb[:, j, 512:768],
                        start=(j == 0), stop=(j == HPC - 1))
                ot = opool.tile([128, D], F32, tag="o", name="ot")
                nc.vector.tensor_copy(out=ot[:, 0:512], in_=c1)
                nc.vector.tensor_copy(out=ot[:, 512:768], in_=c2)
                nc.sync.dma_start(out=out_d[t * 128:(t + 1) * 128, :], in_=ot)




def _emit_v2(nc, tc, xT_d, wp_d, bp_d, wo_d, out_d, exp_group=4):
    """Per-head pipeline; S^T psum in bf16 when exp_group=4 (2048-wide exp)."""
    import contextlib
    ctx = contextlib.ExitStack()
    with ctx:
        wpool = ctx.enter_context(tc.tile_pool(name="wpool", bufs=1))
        persist = ctx.enter_context(tc.tile_pool(name="persist", bufs=1))
        hpool = ctx.enter_context(tc.tile_pool(name="hpool", bufs=2))
        xpool = ctx.enter_context(tc.tile_pool(name="xpool", bufs=2))
        epool = ctx.enter_context(tc.tile_pool(name="epool", bufs=4))
        rpool = ctx.enter_context(tc.tile_pool(name="rpool", bufs=1))
        opool = ctx.enter_context(tc.tile_pool(name="opool", bufs=2))
        # one shared PSUM pool for S-groups/proj/transposes/phase C
        # (3 slots of 2 banks) + the two O accumulators (1 bank each)
        ppS = ctx.enter_context(tc.tile_pool(name="ppS", bufs=3, space="PSUM"))
        ppO = ctx.enter_context(tc.tile_pool(name="ppO", bufs=1, space="PSUM"))
        ppA = ppS

        SDT = BF16 if exp_group == 4 else F32
        NG = NKB // exp_group

        wsb = wpool.tile([128, HPC, 2, 6, 128], F32R)
        nc.sync.dma_start(out=wsb, in_=wp_d.rearrange("j g c p m -> p j g c m"))
        bsb = wpool.tile([128, HPC, 2], F32)
        nc.sync.dma_start(out=bsb, in_=bp_d)
        wosb = wpool.tile([DK, HPC, D], F32R)
        nc.sync.dma_start(out=wosb, in_=wo_d.rearrange("j d m -> d j m"))
        ident = wpool.tile([128, 128], BF16)
        make_identity(nc, ident)

        OT = [persist.tile([DK + 1, S], OT_DT, tag=f"ot{j}", name=f"ot{j}")
              for j in range(HPC)]

        def emit_c(cqi):
            for t in range(cqi * QC // 128, (cqi + 1) * QC // 128):
                c1 = ppO.tile([128, 512], F32, tag="oa", name="c1")
                c2 = ppO.tile([128, 256], F32, tag="ob", name="c2")
                for jj in range(HPC):
                    nc.tensor.matmul(
                        c1, lhsT=OT[jj][0:DK, t * 128:(t + 1) * 128],
                        rhs=wosb[:, jj, 0:512],
                        start=(jj == 0), stop=(jj == HPC - 1))
                for jj in range(HPC):
                    nc.tensor.matmul(
                        c2, lhsT=OT[jj][0:DK, t * 128:(t + 1) * 128],
                        rhs=wosb[:, jj, 512:768],
                        start=(jj == 0), stop=(jj == HPC - 1))
                ot = opool.tile([128, D], F32, tag="o", name="ot")
                nc.vector.tensor_copy(out=ot[:, 0:512], in_=c1)
                nc.vector.tensor_copy(out=ot[:, 512:768], in_=c2)
                nc.sync.dma_start(
                    out=out_d[t * 128:(t + 1) * 128, :], in_=ot)

        n_xch = S // XCH
        for j in range(HPC):
            # ---- phase A for head j ----
            QT = hpool.tile([128, S], BF16, tag="qt", name="qt")
            KT = hpool.tile([128, S], BF16, tag="kt", name="kt")
            VT = hpool.tile([128, S], BF16, tag="vt", name="vt")
            V = hpool.tile([128, NKB, DK + 1], BF16, tag="v", name="v")
            nc.vector.memset(V[:, :, DK], 1.0)
            for ci in range(n_xch):
                xq = xpool.tile([128, 6, XCH], F32R, tag="x", name="xq")
                nc.sync.dma_start(
                    out=xq,
                    in_=xT_d.rearrange("(c p) q -> p c q", p=128)[
                        :, :, ci * XCH:(ci + 1) * XCH])
                cs = slice(ci * XCH, (ci + 1) * XCH)
                # group 0: (Q | K)
                ps = ppA.tile([128, XCH], F32, tag="s", name="ps")
                for c in range(6):
                    nc.tensor.matmul(
                        ps, lhsT=wsb[:, j, 0, c, :], rhs=xq[:, c, :],
                        start=(c == 0), stop=(c == 5))
                nc.vector.tensor_scalar_add(
                    out=QT[0:64, cs], in0=ps[0:64, :],
                    scalar1=bsb[0:64, j, 0:1])
                nc.vector.tensor_scalar_add(
                    out=KT[64:128, cs], in0=ps[64:128, :],
                    scalar1=bsb[64:128, j, 0:1])
                # group 1: (V | V) duplicated
                ps2 = ppA.tile([128, XCH], F32, tag="s", name="ps2")
                for c in range(6):
                    nc.tensor.matmul(
                        ps2, lhsT=wsb[:, j, 1, c, :], rhs=xq[:, c, :],
                        start=(c == 0), stop=(c == 5))
                nc.vector.tensor_scalar_add(
                    out=VT[:, cs], in0=ps2, scalar1=bsb[:, j, 1:2])
                # V natural layout via PE transposes (chunk's k-blocks)
                for kb in range(ci * XCH // 128, (ci + 1) * XCH // 128):
                    pt = ppA.tile([128, 128], BF16, tag="s", name="pt")
                    nc.tensor.transpose(
                        pt, VT[:, kb * 128:(kb + 1) * 128], ident)
                    nc.vector.tensor_copy(
                        out=V[:, kb, 0:DK], in_=pt[:, 0:DK])
            # duplicate halves: Q lower->upper, K upper->lower
            nc.sync.dma_start(out=QT[64:128, :], in_=QT[0:64, :])
            nc.sync.dma_start(out=KT[0:64, :], in_=KT[64:128, :])

            # ---- phase B for head j ----
            for qi in range(NQC):
                qs = qi * QC
                poa = ppO.tile([DK + 1, QC], F32, tag="oa", name="poa")
                pob = ppO.tile([DK + 1, QC], F32, tag="ob", name="pob")
                for g in range(NG):
                    pss = ppS.tile([128, exp_group, QC], SDT, tag="s",
                                   name="pss")
                    # T0 row-tile: first half of the group's k-blocks;
                    # T8: second half (separate PSUM banks)
                    hg = exp_group // 2
                    for i in range(hg):
                        kb = g * exp_group + i
                        nc.tensor.matmul(
                            pss[:, i, :],
                            lhsT=KT[0:64, kb * 128:(kb + 1) * 128],
                            rhs=QT[0:64, qs:qs + QC],
                            start=True, stop=True)
                    for i in range(hg):
                        kb = g * exp_group + hg + i
                        nc.tensor.matmul(
                            pss[:, hg + i, :],
                            lhsT=KT[64:128, kb * 128:(kb + 1) * 128],
                            rhs=QT[64:128, qs:qs + QC],
                            start=True, stop=True)
                    es = epool.tile([128, exp_group, QC], BF16, tag="e",
                                    name="es")
                    nc.scalar.activation(
                        out=es, in_=pss,
                        func=mybir.ActivationFunctionType.Exp, scale=0.125)
                    for s in range(exp_group):
                        kb = g * exp_group + s
                        first = g == 0 and s == 0
                        last = g == NG - 1 and s == exp_group - 1
                        nc.tensor.matmul(
                            poa, lhsT=V[0:64, kb, :], rhs=es[0:64, s, :],
                            start=first, stop=last, skip_group_check=True)
                        nc.tensor.matmul(
                            pob, lhsT=V[64:128, kb, :], rhs=es[64:128, s, :],
                            start=first, stop=last, skip_group_check=True)
                nc.vector.tensor_copy(out=OT[j][:, qs:qs + QC], in_=poa)
                nc.vector.tensor_add(
                    out=OT[j][:, qs:qs + QC],
                    in0=OT[j][:, qs:qs + QC], in1=pob)
                nc.vector.reciprocal(
                    out=OT[j][DK:DK + 1, qs:qs + QC],
                    in_=OT[j][DK:DK + 1, qs:qs + QC])
                srow = rpool.tile([1, QC], OT_DT, tag="sr", name="srow")
                nc.sync.dma_start(
                    out=srow, in_=OT[j][DK:DK + 1, qs:qs + QC])
                rbc = rpool.tile([DK + 1, QC], OT_DT, tag="r", name="rbc")
                nc.gpsimd.partition_broadcast(rbc, srow, channels=DK + 1)
                nc.vector.tensor_mul(
                    out=OT[j][0:DK, qs:qs + QC],
                    in0=OT[j][0:DK, qs:qs + QC], in1=rbc[0:DK, :])

        # ---- phase C: output projection (borrows psumO slots) ----
        for cqi in range(NQC):
            emit_c(cqi)




# Schraudolph fast-exp on DVE: i16 = round(s*A + B); bitcast to bf16
# approximates exp(0.125*s) with ~3% max rel err. Groups listed in
# DVE_EXP_GROUPS (of the 16 exp-groups per q-chunk) use it; rest use ACT.
SCH_A = 0.125 * np.log2(np.e) * 128.0
SCH_B = 127.0 * 128.0 - 4.8
DVE_EXP_GROUPS = frozenset((1, 4, 7, 10, 13))
# which of the 16 exp-groups per q-chunk use the DVE fast exp, per mode
DVE_PATTERNS = {
    "v3s": frozenset((1, 4, 7, 10, 13)),
    "v3ps": frozenset((1, 4, 7, 10, 13)),
    "v3ps5": frozenset(range(1, 16, 2)),
    "v3ps6": frozenset((1, 2, 4, 5, 7, 8, 10, 11, 13, 14)),
}


def _emit_v3(nc, tc, xT_d, wp_d, bp_d, wo_d, out_d, dve_exp=False,
             no_c=False, no_o=False, no_exp=False, packed=False,
             exp4=False, dve_groups=frozenset(), early=False,
             split_o=False, ot_bf16=False):
    """v3: full-contraction O matmuls (single accumulator), separate PSUM
    pools per phase, software-pipelined exp->O, and phase A(j+1)/C emission
    interleaved into phase B(j) q-chunks.

    packed (v3p): head 0's V-projection group computes (V0|V1) so head 1
    needs no V group and one transpose serves two heads; head 2 keeps a
    duplicated (V2|V2) group. VT becomes a per-chunk transient tile.

    no_c/no_o/no_exp are timing-only ablations (wrong results)."""
    import contextlib
    ctx = contextlib.ExitStack()
    with ctx:
        wpool = ctx.enter_context(tc.tile_pool(name="wpool", bufs=1))
        otpool = ctx.enter_context(tc.tile_pool(name="otpool", bufs=1))
        hpool = ctx.enter_context(tc.tile_pool(name="hpool", bufs=2))
        xpool = ctx.enter_context(tc.tile_pool(name="xpool", bufs=2))
        epool = ctx.enter_context(tc.tile_pool(name="epool", bufs=4))
        rpool = ctx.enter_context(tc.tile_pool(name="rpool", bufs=1))
        opool = ctx.enter_context(tc.tile_pool(name="opool", bufs=2))
        vtpool = ctx.enter_context(tc.tile_pool(name="vtpool", bufs=2))
        # PSUM budget (8 banks): S-score slots 2x2, O-accum/phase-C 2x1,
        # phase-A proj/transpose 2x1
        ppS = ctx.enter_context(tc.tile_pool(name="ppS", bufs=2, space="PSUM"))
        ppO = ctx.enter_context(tc.tile_pool(name="ppO", bufs=2, space="PSUM"))
        ppA = ctx.enter_context(
            tc.tile_pool(name="ppA", bufs=1 if split_o else 2, space="PSUM"))
        ppOb = ctx.enter_context(
            tc.tile_pool(name="ppOb", bufs=1, space="PSUM")) if split_o \
            else None

        wsb = wpool.tile([128, HPC, 2, 6, 128], F32R)
        nc.sync.dma_start(out=wsb, in_=wp_d.rearrange("j g c p m -> p j g c m"))
        bsb = wpool.tile([128, HPC, 2], F32)
        nc.sync.dma_start(out=bsb, in_=bp_d)
        if ot_bf16:
            wosf = wpool.tile([DK, HPC, D], F32R, tag="wof", name="wosf")
            nc.sync.dma_start(out=wosf, in_=wo_d.rearrange("j d m -> d j m"))
            wosb = wpool.tile([DK, HPC, D], BF16, tag="wo", name="wosb")
            nc.vector.tensor_copy(out=wosb, in_=wosf)
        else:
            wosb = wpool.tile([DK, HPC, D], F32R)
            nc.sync.dma_start(out=wosb, in_=wo_d.rearrange("j d m -> d j m"))
        ident = wpool.tile([128, 128], BF16)
        make_identity(nc, ident)

        odt = BF16 if ot_bf16 else OT_DT
        OT = [otpool.tile([DK + 1, S], odt, tag=f"ot{j}", name=f"ot{j}")
              for j in range(HPC)]

        es_const = None
        if no_exp:
            es_const = wpool.tile([128, 2, QC], BF16, tag="ec", name="ec")
            nc.vector.memset(es_const, 0.001)

        n_xch = S // XCH

        def alloc_head(j):
            t = {
                "QT": hpool.tile([128, S], BF16, tag="qt", name="qt"),
                "KT": hpool.tile([128, S], BF16, tag="kt", name="kt"),
                "V": hpool.tile([128, NKB, DK + 1], BF16, tag="v", name="v"),
            }
            if not packed:
                t["VT"] = hpool.tile([128, S], BF16, tag="vt", name="vt")
            nc.vector.memset(t["V"][:, :, DK], 1.0)
            return t

        def emit_A(j, t, ci, v_dsts=None):
            QT, KT = t["QT"], t["KT"]
            if v_dsts is None:
                v_dsts = [(t["V"], 0)]
            xq = xpool.tile([128, 6, XCH], F32R, tag="x", name="xq")
            nc.sync.dma_start(
                out=xq,
                in_=xT_d.rearrange("(c p) q -> p c q", p=128)[
                    :, :, ci * XCH:(ci + 1) * XCH])
            cs = slice(ci * XCH, (ci + 1) * XCH)
            # group 0: (Q | K)
            ps = ppA.tile([128, XCH], F32, tag="a", name="ps")
            for c in range(6):
                nc.tensor.matmul(
                    ps, lhsT=wsb[:, j, 0, c, :], rhs=xq[:, c, :],
                    start=(c == 0), stop=(c == 5))
            nc.vector.tensor_scalar_add(
                out=QT[0:64, cs], in0=ps[0:64, :], scalar1=bsb[0:64, j, 0:1])
            nc.vector.tensor_scalar_add(
                out=KT[64:128, cs], in0=ps[64:128, :],
                scalar1=bsb[64:128, j, 0:1])
            # duplicate halves per chunk (Q lower->upper, K upper->lower)
            nc.sync.dma_start(out=QT[64:128, cs], in_=QT[0:64, cs])
            nc.sync.dma_start(out=KT[0:64, cs], in_=KT[64:128, cs])
            if not v_dsts:
                return
            # group 1: V halves (either (Vj|Vj) dup'd or (Vj|Vj+1) packed)
            ps2 = ppA.tile([128, XCH], F32, tag="a", name="ps2")
            for c in range(6):
                nc.tensor.matmul(
                    ps2, lhsT=wsb[:, j, 1, c, :], rhs=xq[:, c, :],
                    start=(c == 0), stop=(c == 5))
            if packed:
                VT = vtpool.tile([128, XCH], BF16, tag="vt", name="vtc")
                vcs = slice(0, XCH)
            else:
                VT = t["VT"]
                vcs = cs
            nc.vector.tensor_scalar_add(
                out=VT[:, vcs], in0=ps2, scalar1=bsb[:, j, 1:2])
            # V natural layout via PE transposes; all 4 blocks land in one
            # psum tile so one strided DVE copy evacuates them per head
            pt4 = ppA.tile([128, XCH // 128, 128], BF16, tag="a", name="pt4")
            for kk in range(XCH // 128):
                lo = (0 if packed else ci * XCH) + kk * 128
                nc.tensor.transpose(pt4[:, kk, :], VT[:, lo:lo + 128], ident)
            kb0 = ci * XCH // 128
            for Vt, off in v_dsts:
                nc.vector.tensor_copy(
                    out=Vt[:, kb0:kb0 + XCH // 128, 0:DK],
                    in_=pt4[:, :, off:off + DK])

        def b_chunk_gen(j, t, qi):
            QT, KT, V = t["QT"], t["KT"], t["V"]
            qs = qi * QC
            poa = ppO.tile([DK + 1, QC], F32, tag="oa", name="poa")
            pob = ppOb.tile([DK + 1, QC], F32, tag="ob", name="pob") \
                if split_o else None

            GW = 4 if exp4 else 2  # k-blocks per exp group

            def emit_O(es, g):
                if no_o:
                    if g == 0:
                        nc.vector.memset(poa, 1.0)
                    return
                for s in range(GW):
                    kb = GW * g + s
                    st, sp = (kb == 0), (kb == NKB - 1)
                    if split_o:
                        # half-contraction pair on disjoint row groups: the
                        # two matmuls overlap on the PE array
                        nc.tensor.matmul(
                            poa, lhsT=V[0:64, kb, :], rhs=es[0:64, s, :],
                            start=st, stop=sp, skip_group_check=True)
                        nc.tensor.matmul(
                            pob, lhsT=V[64:128, kb, :], rhs=es[64:128, s, :],
                            start=st, stop=sp, skip_group_check=True)
                    else:
                        nc.tensor.matmul(
                            poa, lhsT=V[:, kb, :], rhs=es[:, s, :],
                            start=st, stop=sp, skip_group_check=True)

            # S-matmul emission order alternates row halves so consecutive
            # matmuls hit disjoint row groups
            s_order = ((0, 0), (2, 1), (1, 0), (3, 1)) if exp4 else \
                ((0, 0), (1, 1))
            pend = None
            for g in range(NKB // GW):
                pss = ppS.tile([128, GW, QC], FP16 if exp4 else F32,
                               tag="s", name="pss")
                for idx, half in s_order:
                    kb = GW * g + idx
                    lo = 64 * half
                    nc.tensor.matmul(
                        pss[:, idx, :],
                        lhsT=KT[lo:lo + 64, kb * 128:(kb + 1) * 128],
                        rhs=QT[lo:lo + 64, qs:qs + QC],
                        start=True, stop=True)
                if no_exp:
                    es = es_const
                elif (dve_exp and g in DVE_EXP_GROUPS) or g in dve_groups:
                    esi = epool.tile([128, GW, QC], I16, tag="e", name="esi")
                    nc.vector.tensor_scalar(
                        out=esi, in0=pss, scalar1=SCH_A, scalar2=SCH_B,
                        op0=mybir.AluOpType.mult, op1=mybir.AluOpType.add)
                    es = esi.bitcast(BF16)
                else:
                    es = epool.tile([128, GW, QC], BF16, tag="e", name="es")
                    nc.scalar.activation(
                        out=es, in_=pss,
                        func=mybir.ActivationFunctionType.Exp, scale=0.125)
                if pend is not None:
                    emit_O(*pend)
                pend = (es, g)
                yield
            emit_O(*pend)

            nc.vector.tensor_copy(out=OT[j][:, qs:qs + QC], in_=poa)
            if split_o:
                # DVE may read only one PSUM operand per op
                nc.vector.tensor_add(
                    out=OT[j][:, qs:qs + QC],
                    in0=OT[j][:, qs:qs + QC], in1=pob)
            nc.vector.reciprocal(
                out=OT[j][DK:DK + 1, qs:qs + QC],
                in_=OT[j][DK:DK + 1, qs:qs + QC])
            srow = rpool.tile([1, QC], odt, tag="sr", name="srow")
            nc.sync.dma_start(out=srow, in_=OT[j][DK:DK + 1, qs:qs + QC])
            rbc = rpool.tile([DK + 1, QC], odt, tag="r", name="rbc")
            nc.gpsimd.partition_broadcast(rbc, srow, channels=DK + 1)
            nc.vector.tensor_mul(
                out=OT[j][0:DK, qs:qs + QC],
                in0=OT[j][0:DK, qs:qs + QC], in1=rbc[0:DK, :])

        def emit_B_chunk(j, t, qi):
            for _ in b_chunk_gen(j, t, qi):
                pass

        def emit_C_range(qi):
            if no_c:
                return
            for ti in range(qi * QC // 128, (qi + 1) * QC // 128):
                c1 = ppO.tile([128, 512], F32, tag="oa", name="c1")
                for jj in range(HPC):
                    nc.tensor.matmul(
                        c1, lhsT=OT[jj][0:DK, ti * 128:(ti + 1) * 128],
                        rhs=wosb[:, jj, 0:512],
                        start=(jj == 0), stop=(jj == HPC - 1))
                c2 = ppO.tile([128, 256], F32, tag="oa", name="c2")
                for jj in range(HPC):
                    nc.tensor.matmul(
                        c2, lhsT=OT[jj][0:DK, ti * 128:(ti + 1) * 128],
                        rhs=wosb[:, jj, 512:768],
                        start=(jj == 0), stop=(jj == HPC - 1))
                ot = opool.tile([128, D], F32, tag="o", name="ot")
                nc.vector.tensor_copy(out=ot[:, 0:512], in_=c1)
                nc.vector.tensor_copy(out=ot[:, 512:768], in_=c2)
                nc.sync.dma_start(
                    out=out_d[ti * 128:(ti + 1) * 128, :], in_=ot)

        # prologue: head 0 phase A, optionally with B(0, qi=0) groups
        # spliced in (each A chunk readies 4 k-blocks = 2 exp groups,
        # lagging one chunk)
        heads = [alloc_head(0)]
        if packed:
            heads.append(alloc_head(1))
        pro_v = [(heads[0]["V"], 0), (heads[1]["V"], 64)] if packed else None
        b0 = b_chunk_gen(0, heads[0], 0) if early else None
        for ci in range(n_xch):
            emit_A(0, heads[0], ci, v_dsts=pro_v)
            if early and ci >= 1:
                next(b0)
                next(b0)
        if early:
            for _ in b0:
                pass
        for j in range(HPC):
            t = heads[j]
            if j + 1 < HPC and len(heads) == j + 1:
                heads.append(alloc_head(j + 1))
            q0 = 1 if (early and j == 0) else 0
            for qi in range(q0, NQC):
                emit_B_chunk(j, t, qi)
                if j + 1 < HPC:
                    if packed:
                        v_dsts = [] if j == 0 else [(heads[2]["V"], 0)]
                        emit_A(j + 1, heads[j + 1], qi - q0, v_dsts=v_dsts)
                    else:
                        emit_A(j + 1, heads[j + 1], qi - q0)
                elif qi > 0:
                    # lag C by one q-chunk so its psum-slot reuse and the
                    # OT normalization latency chain decouple from B's
                    # pipeline
                    emit_C_range(qi - 1)
            if j + 1 < HPC:
                for ci in range(NQC - q0, NQC):
                    if packed:
                        v_dsts = [] if j == 0 else [(heads[2]["V"], 0)]
                        emit_A(j + 1, heads[j + 1], ci, v_dsts=v_dsts)
                    else:
                        emit_A(j + 1, heads[j + 1], ci)
        emit_C_range(NQC - 1)


def _emit_mb(nc, tc, out_d, paired=True, n_slots=1024):
    """Microbenchmark: 2048 64-contraction matmuls (N=512), either as
    row-half pairs writing separate banks (paired) or serial
    full-contraction (unpaired: 2048 full-row MMs). Measures whether
    row-tiled MMs overlap on this hardware."""
    import contextlib
    ctx = contextlib.ExitStack()
    with ctx:
        pool = ctx.enter_context(tc.tile_pool(name="mb", bufs=1))
        opool = ctx.enter_context(tc.tile_pool(name="mbo", bufs=2))
        pp = ctx.enter_context(tc.tile_pool(name="mbp", bufs=2, space="PSUM"))
        KT = pool.tile([128, S], BF16, tag="kt", name="kt")
        QT = pool.tile([128, QC], BF16, tag="qt", name="qt")
        nc.vector.memset(KT, 0.01)
        nc.vector.memset(QT, 0.01)
        NACC = 64  # matmuls accumulated per psum bank before evacuation
        for rep in range(n_slots // NACC):
            ps = pp.tile([128, 2, QC], F32, tag="s", name="ps")
            for i in range(NACC):
                kb = (rep * NACC + i) % NKB
                st, sp = (i == 0), (i == NACC - 1)
                if paired:
                    nc.tensor.matmul(
                        ps[:, 0, :], lhsT=KT[0:64, kb * 128:(kb + 1) * 128],
                        rhs=QT[0:64, :], start=st, stop=sp,
                        skip_group_check=True)
                    nc.tensor.matmul(
                        ps[:, 1, :], lhsT=KT[64:128, kb * 128:(kb + 1) * 128],
                        rhs=QT[64:128, :], start=st, stop=sp,
                        skip_group_check=True)
                else:
                    nc.tensor.matmul(
                        ps[:, 0, :], lhsT=KT[:, kb * 128:(kb + 1) * 128],
                        rhs=QT, start=st, stop=sp, skip_group_check=True)
                    nc.tensor.matmul(
                        ps[:, 1, :], lhsT=KT[:, kb * 128:(kb + 1) * 128],
                        rhs=QT, start=st, stop=sp, skip_group_check=True)
            ot = opool.tile([128, 2, QC], F32, tag="o", name="ot")
            nc.vector.tensor_copy(out=ot, in_=ps)
            if rep == 0:
                nc.sync.dma_start(
                    out=out_d[0:128, 0:QC], in_=ot[:, 0, :])


# ---------------------------------------------------------------------------
# host side
# ---------------------------------------------------------------------------

KERNEL_MODE = "v3pe"


def shard_inputs(x, Wq, bq, Wk, bk, Wv, bv, Wo, bo, mode=None):
    """Build the 8 per-core input maps."""
    mode = mode or KERNEL_MODE
    if mode.startswith("v5") or (mode.startswith("v3") and "p" in mode):
        return shard_inputs_v3p(x, Wq, bq, Wk, bk, Wv, bv, Wo, bo)
    if mode.startswith("v2") or mode.startswith("v3"):
        return shard_inputs_v2(x, Wq, bq, Wk, bk, Wv, bv, Wo, bo)
    return shard_inputs_v1(x, Wq, bq, Wk, bk, Wv, bv, Wo, bo)


def shard_inputs_v3p(x, Wq, bq, Wk, bk, Wv, bv, Wo, bo):
    """v3p packing: wp[0,1] = (V0|V1), wp[1,1] unused, wp[2,1] = (V2|V2)."""
    x = np.asarray(x, np.float32)
    Wq, Wk, Wv = (np.asarray(a, np.float32) for a in (Wq, Wk, Wv))
    bq, bk, bv = (np.asarray(a, np.float32) for a in (bq, bk, bv))
    Wo = np.asarray(Wo, np.float32)
    in_maps = []
    for c in range(N_CORES):
        b, g = divmod(c, 4)
        heads = [3 * g + j for j in range(HPC)]
        wp = np.zeros((HPC, 2, 6, 128, 128), np.float32)
        bp = np.zeros((128, HPC, 2), np.float32)
        wo = np.empty((HPC, DK, D), np.float32)
        for j, h in enumerate(heads):
            sl = slice(64 * h, 64 * h + 64)
            wp[j, 0, :, :, 0:64] = Wq[sl].T.reshape(6, 128, 64)
            wp[j, 0, :, :, 64:128] = Wk[sl].T.reshape(6, 128, 64)
            bp[0:64, j, 0] = bq[sl]
            bp[64:128, j, 0] = bk[sl]
            wo[j] = Wo[:, sl].T
        # V groups: head0 -> (V0|V1); head2 -> (V2|V2)
        h0, h1, h2 = heads
        for (jj, half, h) in ((0, 0, h0), (0, 1, h1), (2, 0, h2), (2, 1, h2)):
            sl = slice(64 * h, 64 * h + 64)
            wp[jj, 1, :, :, 64 * half:64 * half + 64] = \
                Wv[sl].T.reshape(6, 128, 64)
            bp[64 * half:64 * half + 64, jj, 1] = bv[sl]
        in_maps.append({
            "xT": np.ascontiguousarray(x[b].T),
            "wp": wp, "bp": bp, "wo": wo,
        })
    return in_maps


def shard_inputs_v2(x, Wq, bq, Wk, bk, Wv, bv, Wo, bo):
    x = np.asarray(x, np.float32)
    Wq, Wk, Wv = (np.asarray(a, np.float32) for a in (Wq, Wk, Wv))
    bq, bk, bv = (np.asarray(a, np.float32) for a in (bq, bk, bv))
    Wo = np.asarray(Wo, np.float32)
    in_maps = []
    for c in range(N_CORES):
        b, g = divmod(c, 4)
        heads = [3 * g + j for j in range(HPC)]
        wp = np.empty((HPC, 2, 6, 128, 128), np.float32)
        bp = np.zeros((128, HPC, 2), np.float32)
        wo = np.empty((HPC, DK, D), np.float32)
        for j, h in enumerate(heads):
            sl = slice(64 * h, 64 * h + 64)
            wp[j, 0, :, :, 0:64] = Wq[sl].T.reshape(6, 128, 64)
            wp[j, 0, :, :, 64:128] = Wk[sl].T.reshape(6, 128, 64)
            wp[j, 1, :, :, 0:64] = Wv[sl].T.reshape(6, 128, 64)
            wp[j, 1, :, :, 64:128] = Wv[sl].T.reshape(6, 128, 64)
            bp[0:64, j, 0] = bq[sl]
            bp[64:128, j, 0] = bk[sl]
            bp[0:64, j, 1] = bv[sl]
            bp[64:128, j, 1] = bv[sl]
            wo[j] = Wo[:, sl].T
        in_maps.append({
            "xT": np.ascontiguousarray(x[b].T),
            "wp": wp, "bp": bp, "wo": wo,
        })
    return in_maps


def shard_inputs_v1(x, Wq, bq, Wk, bk, Wv, bv, Wo, bo):
    """Build the 8 per-core input maps."""
    x = np.asarray(x, np.float32)
    Ws = {0: np.asarray(Wq, np.float32), 1: np.asarray(Wk, np.float32),
          2: np.asarray(Wv, np.float32)}
    bs = {0: np.asarray(bq, np.float32), 1: np.asarray(bk, np.float32),
          2: np.asarray(bv, np.float32)}
    Wo = np.asarray(Wo, np.float32)
    in_maps = []
    for c in range(N_CORES):
        b, g = divmod(c, 4)
        heads = [3 * g + j for j in range(HPC)]
        wp = np.empty((5, 6, 128, 128), np.float32)
        bp = np.zeros((128, 5), np.float32)
        for gi, (mA, mB) in enumerate(PROJ_GROUPS):
            for half, (j, kind) in ((0, mA), (1, mB)):
                h = heads[j]
                Wh = Ws[kind][64 * h:64 * h + 64, :]       # [64, 768]
                chunks = Wh.T.reshape(6, 128, 64)          # [c, p, 64]
                wp[gi, :, :, half * 64:half * 64 + 64] = chunks
                bp[half * 64:half * 64 + 64, gi] = bs[kind][64 * h:64 * h + 64]
        wo = np.empty((HPC, DK, D), np.float32)
        for j in range(HPC):
            h = heads[j]
            wo[j] = Wo[:, 64 * h:64 * h + 64].T
        in_maps.append({
            "xT": np.ascontiguousarray(x[b].T),
            "wp": wp, "bp": bp, "wo": wo,
        })
    return in_maps


def assemble_output(parts, bo):
    out = np.empty((B, S, D), np.float32)
    for b in range(B):
        acc = parts[4 * b]["out"].astype(np.float32).copy()
        for c in range(4 * b + 1, 4 * b + 4):
            acc += parts[c]["out"]
        out[b] = acc + np.asarray(bo, np.float32)[None, :]
    return out


_RUNNER = None


def _make_runner(nc):
    """Reusable PJRT runner (mirrors bass2jax.run_bass_via_pjrt multi-core)."""
    import jax
    import jax.numpy as jnp
    from jax.experimental.shard_map import shard_map
    from jax.sharding import Mesh, PartitionSpec
    from concourse import bass2jax

    bass2jax.install_neuronx_cc_hook()

    partition_name = (nc.partition_id_tensor.name
                      if nc.partition_id_tensor else None)
    in_names, out_names, out_avals = [], [], []
    for alloc in nc.m.functions[0].allocations:
        if not isinstance(alloc, mybir.MemoryLocationSet):
            continue
        name = alloc.memorylocations[0].name
        if alloc.kind == "ExternalInput":
            if name != partition_name:
                in_names.append(name)
        elif alloc.kind == "ExternalOutput":
            out_names.append(name)
            out_avals.append(jax.core.ShapedArray(
                tuple(alloc.tensor_shape), mybir.dt.np(alloc.dtype)))
    n_params = len(in_names)
    n_outs = len(out_names)
    all_in_names = list(in_names) + list(out_names)
    if partition_name is not None:
        all_in_names.append(partition_name)
    donate = tuple(range(n_params, n_params + n_outs))

    def _body(*args):
        operands = list(args)
        if partition_name is not None:
            operands.append(bass2jax.partition_id_tensor())
        outs = bass2jax._bass_exec_p.bind(
            *operands,
            out_avals=tuple(out_avals),
            in_names=tuple(all_in_names),
            out_names=tuple(out_names),
            lowering_input_output_aliases=(),
            sim_require_finite=True,
            sim_require_nnan=True,
            nc=nc,
        )
        return tuple(outs)

    devices = jax.devices()[:N_CORES]
    mesh = Mesh(np.asarray(devices), ("core",))
    in_specs = (PartitionSpec("core"),) * (n_params + n_outs)
    out_specs = (PartitionSpec("core"),) * n_outs
    sharded = jax.jit(
        shard_map(_body, mesh=mesh, in_specs=in_specs, out_specs=out_specs,
                  check_rep=False),
        donate_argnums=donate, keep_unused=True)

    def run(in_maps):
        per_core = [[np.asarray(m[name]) for name in in_names]
                    for m in in_maps]
        concat_in = [np.concatenate([per_core[c][i] for c in range(N_CORES)],
                                    axis=0) for i in range(n_params)]
        zeros = [np.zeros((N_CORES * av.shape[0], *av.shape[1:]), av.dtype)
                 for av in out_avals]
        outs = sharded(*concat_in, *zeros)
        return [
            {name: np.asarray(outs[i]).reshape(N_CORES, *out_avals[i].shape)[c]
             for i, name in enumerate(out_names)}
            for c in range(N_CORES)
        ]

    run.sharded = sharded
    run.in_names = in_names
    run.out_names = out_names
    run.out_avals = out_avals
    run.n_params = n_params
    return run


def get_runner():
    global _RUNNER
    if _RUNNER is None:
        nc = build_program()
        _RUNNER = _make_runner(nc)
    return _RUNNER


def kernel(x, Wq, bq, Wk, bk, Wv, bv, Wo, bo):
    run = get_runner()
    in_maps = shard_inputs(x, Wq, bq, Wk, bk, Wv, bv, Wo, bo)
    parts = run(in_maps)
    return assemble_output(parts, bo)



# revision 60
# speedup vs baseline: 1.4268x; 1.1031x over previous
"""Multi-head attention (B=2, S=4096, D=768, H=12, d_k=64) on 8 TRN2 cores.

Sharding: core c -> batch b = c//4, head group g = c%4 (heads 3g..3g+2).
Each core computes partial = sum_{h in group} softmax(QK^T/8) V @ Wo_h^T
over its batch; host sums the 4 partials per batch and adds bo.

Device kernel (identical SPMD program, per-core data):
  Phase A: QKV projections (fp32r matmuls), Q^T/K^T/V^T produced in
           [head_dim, seq] layout (bf16), V transposed to natural
           [seq, head_dim] layout with a ones column appended (row sums).
  Phase B: per (head, q-chunk of 512): S^T tiles [128k, 512q] via
           64-contraction matmuls (two concurrent row-tiles T0/T8),
           exp on ACT from 2-bank PSUM groups -> bf16, O^T accumulation
           with V|ones (row 64 = softmax sums), per-q normalization via
           reciprocal + gpsimd partition broadcast.
  Phase C: out[qtile] = sum_h O_h^T.T @ Wo_h^T (fp32r), DMA to DRAM.
"""

import numpy as np

import concourse.bass as bass
import concourse.mybir as mybir
import concourse.tile as tile
from concourse import bacc
from concourse.masks import make_identity

F32 = mybir.dt.float32
FP16 = mybir.dt.float16
I16 = mybir.dt.int16
F32R = mybir.dt.float32r
BF16 = mybir.dt.bfloat16

N_CORES = 8
B, S, D = 2, 4096, 768
H, DK = 12, 64
HPC = 3            # heads per core
QC = 512           # q-chunk width (free dim of S^T matmuls)
NQC = S // QC      # 8
NKB = S // 128     # 32 k-blocks of 128
XCH = 512          # x streaming chunk (columns of x^T per DMA)
OT_DT = F32R       # dtype of O^T staging

# projection group packing: 5 groups of two 64-dim tensors (by (head, kind))
# kind: 0=Q, 1=K, 2=V
PROJ_GROUPS = [((0, 0), (0, 1)), ((0, 2), (1, 0)), ((1, 1), (1, 2)),
               ((2, 0), (2, 1)), ((2, 2), (2, 2))]


def build_program(debug=False, repeat=1, mode=None):
    mode = mode or KERNEL_MODE
    nc = bacc.Bacc("TRN2", debug=False, num_devices=N_CORES)

    xT_d = nc.dram_tensor("xT", [D, S], F32R, kind="ExternalInput").ap()
    if mode.startswith(("v2", "v3", "v4", "v5")):
        wp_d = nc.dram_tensor("wp", [HPC, 2, 6, 128, 128], F32R,
                              kind="ExternalInput").ap()
        bp_d = nc.dram_tensor("bp", [128, HPC, 2], F32,
                              kind="ExternalInput").ap()
    else:
        wp_d = nc.dram_tensor("wp", [5, 6, 128, 128], F32R,
                              kind="ExternalInput").ap()
        bp_d = nc.dram_tensor("bp", [128, 5], F32, kind="ExternalInput").ap()
    wo_d = nc.dram_tensor("wo", [HPC, DK, D], F32R, kind="ExternalInput").ap()
    out_d = nc.dram_tensor("out", [S, D], F32, kind="ExternalOutput").ap()

    dbg = {}
    if debug:
        dbg["qt"] = nc.dram_tensor("d_qt", [128, S], BF16,
                                   kind="ExternalOutput").ap()
        dbg["kt"] = nc.dram_tensor("d_kt", [128, S], BF16,
                                   kind="ExternalOutput").ap()
        dbg["v"] = nc.dram_tensor("d_v", [128, NKB, DK + 1], BF16,
                                  kind="ExternalOutput").ap()
        dbg["es"] = nc.dram_tensor("d_es", [128, 2, QC], BF16,
                                   kind="ExternalOutput").ap()
        dbg["po"] = nc.dram_tensor("d_po", [2, DK + 1, QC], F32,
                                   kind="ExternalOutput").ap()
        dbg["otr"] = nc.dram_tensor("d_otr", [DK + 1, QC], F32,
                                    kind="ExternalOutput").ap()
        dbg["rbc"] = nc.dram_tensor("d_rbc", [DK + 1, QC], F32,
                                    kind="ExternalOutput").ap()
        dbg["ot"] = nc.dram_tensor("d_ot", [DK + 1, S], F32,
                                   kind="ExternalOutput").ap()

    with tile.TileContext(nc) as tc, \
            nc.allow_low_precision("bf16/fp32r attention pipeline"):
        if mode.startswith("mb"):
            for _ in range(repeat):
                _emit_mb(nc, tc, out_d, paired=(mode == "mba"))
        elif mode.startswith(("v3", "v4", "v5")):
            assert not debug and repeat >= 1
            for _ in range(repeat):
                _emit_v3(nc, tc, xT_d, wp_d, bp_d, wo_d, out_d,
                         no_c=("xC" in mode), no_o=("xO" in mode),
                         no_exp=("xE" in mode),
                         packed=("p" in mode or mode.startswith("v5")),
                         early=("e" in mode or mode.startswith("v5")),
                         split_o=mode.startswith("v5"),
                         ot_bf16=("b" in mode),
                         fast_recip=("r" in mode),
                         group3=("g" in mode),
                         dve_groups=DVE_PATTERNS.get(mode, frozenset()))
        elif mode.startswith("v2"):
            assert not debug and repeat >= 1
            for _ in range(repeat):
                _emit_v2(nc, tc, xT_d, wp_d, bp_d, wo_d, out_d,
                         exp_group=4 if mode == "v2_e4" else 2)
        else:
            _emit(nc, tc, xT_d, wp_d, bp_d, wo_d, out_d, dbg,
                  repeat=repeat, mode=mode)
    nc.compile()
    return nc


def _emit(nc, tc, xT_d, wp_d, bp_d, wo_d, out_d, dbg={},
          repeat=1, mode="tiled64"):
    import contextlib
    ctx = contextlib.ExitStack()
    with ctx:
        wpool = ctx.enter_context(tc.tile_pool(name="wpool", bufs=1))
        persist = ctx.enter_context(tc.tile_pool(name="persist", bufs=1))
        xpool = ctx.enter_context(tc.tile_pool(name="xpool", bufs=2))
        epool = ctx.enter_context(tc.tile_pool(name="epool", bufs=3))
        rpool = ctx.enter_context(tc.tile_pool(name="rpool", bufs=1))
        opool = ctx.enter_context(tc.tile_pool(name="opool", bufs=2))
        ppS = ctx.enter_context(tc.tile_pool(name="ppS", bufs=2, space="PSUM"))
        ppO = ctx.enter_context(tc.tile_pool(name="ppO", bufs=1, space="PSUM"))
        ppA = ctx.enter_context(tc.tile_pool(name="ppA", bufs=2, space="PSUM"))

        # ---- constants / weights ----
        wsb = wpool.tile([128, 5, 6, 128], F32R)
        nc.sync.dma_start(out=wsb, in_=wp_d.rearrange("g c p m -> p g c m"))
        bsb = wpool.tile([128, 5], F32)
        nc.sync.dma_start(out=bsb, in_=bp_d)
        wosb = wpool.tile([DK, HPC, D], F32R)
        nc.sync.dma_start(out=wosb, in_=wo_d.rearrange("j d m -> d j m"))
        ident = wpool.tile([128, 128], BF16)
        make_identity(nc, ident)

        assert not (dbg and repeat > 1)
        # which half each (head, kind) tensor is written to by the packed
        # projections, derived from PROJ_GROUPS
        wr_half = {}
        for gi, (mA, mB) in enumerate(PROJ_GROUPS):
            if gi == 4:
                wr_half[mA] = 0  # written to both halves
                continue
            wr_half[mA] = 0
            wr_half[mB] = 1

        for rep in range(repeat):
            # ---- persistent per-head tensors ----
            # QT/KT: [head_dim(64) in both halves (tiled64) or lower half +
            # zero upper (pad128), seq] bf16
            QT = [persist.tile([128, S], BF16, tag=f"qt{j}", name=f"qt{j}")
                  for j in range(HPC)]
            KT = [persist.tile([128, S], BF16, tag=f"kt{j}", name=f"kt{j}")
                  for j in range(HPC)]
            # V natural layout + ones column: [128 part = k%128, kb, 65]
            V = [persist.tile([128, NKB, DK + 1], BF16, tag=f"v{j}",
                              name=f"v{j}") for j in range(HPC)]
            # O^T staging: rows 0..63 = head dims, row 64 = softmax sums
            OT = [persist.tile([DK + 1, S], OT_DT, tag=f"ot{j}",
                               name=f"ot{j}") for j in range(HPC)]
            # VT transient [dims(64) at written half, seq] bf16
            VT = [persist.tile([128, S], BF16, tag=f"vt{j}", name=f"vt{j}")
                  for j in range(HPC)]

            for j in range(HPC):
                nc.vector.memset(V[j][:, :, DK], 1.0)

            def tgt(j, kind):
                return QT[j] if kind == 0 else KT[j] if kind == 1 else VT[j]

            # ---- Phase A: projections, x streamed in contraction-complete
            # column chunks ----
            n_xch = S // XCH
            for ci in range(n_xch):
                xq = xpool.tile([128, 6, XCH], F32R, tag="x", name="xq")
                nc.sync.dma_start(
                    out=xq,
                    in_=xT_d.rearrange("(c p) q -> p c q", p=128)[
                        :, :, ci * XCH:(ci + 1) * XCH],
                )
                for gi, (mA, mB) in enumerate(PROJ_GROUPS):
                    ps = ppA.tile([128, XCH], F32, tag="s", name="ps")
                    for c in range(6):
                        nc.tensor.matmul(
                            ps, lhsT=wsb[:, gi, c, :], rhs=xq[:, c, :],
                            start=(c == 0), stop=(c == 5))
                    # evacuate halves with bias add, cast to bf16
                    if gi == 4:
                        # V2 written to both halves at once (dup'd weights)
                        nc.vector.tensor_scalar_add(
                            out=VT[2][:, ci * XCH:(ci + 1) * XCH],
                            in0=ps, scalar1=bsb[:, gi:gi + 1])
                        continue
                    for half, (j, kind) in ((0, mA), (1, mB)):
                        lo, hi = half * 64, half * 64 + 64
                        nc.vector.tensor_scalar_add(
                            out=tgt(j, kind)[lo:hi, ci * XCH:(ci + 1) * XCH],
                            in0=ps[lo:hi, :],
                            scalar1=bsb[lo:hi, gi:gi + 1])

            # fix up Q/K halves (V^T needs none: transposes read the
            # written half directly)
            for j in range(HPC):
                for kind in (0, 1):
                    t = tgt(j, kind)
                    wh = wr_half[(j, kind)]
                    lo, hi = wh * 64, wh * 64 + 64
                    olo, ohi = 64 - lo, 128 - lo
                    if mode == "tiled64":
                        # duplicate into the other half
                        nc.sync.dma_start(out=t[olo:ohi, :], in_=t[lo:hi, :])
                    else:
                        # data to lower half, zero upper
                        if wh == 1:
                            nc.sync.dma_start(out=t[0:64, :], in_=t[64:128, :])
                        nc.vector.memset(t[64:128, :], 0.0)

            # V: transpose VT [dims, seq] -> natural [seq, dims] per block
            for j in range(HPC):
                voff = wr_half[(j, 2)] * 64
                for kb in range(NKB):
                    pt = ppA.tile([128, 128], BF16, tag="s", name="pt")
                    nc.tensor.transpose(
                        pt, VT[j][:, kb * 128:(kb + 1) * 128], ident)
                    nc.vector.tensor_copy(
                        out=V[j][:, kb, 0:DK], in_=pt[:, voff:voff + DK])

            if dbg:
                nc.sync.dma_start(out=dbg["qt"], in_=QT[0])
                nc.sync.dma_start(out=dbg["kt"], in_=KT[0])
                nc.sync.dma_start(out=dbg["v"], in_=V[0])

            # ---- Phase B: attention per head ----
            for j in range(HPC):
                for qi in range(NQC):
                    qs = qi * QC
                    poa = ppO.tile([DK + 1, QC], F32, tag="oa", name="poa")
                    if mode == "tiled64":
                        pob = ppO.tile([DK + 1, QC], F32, tag="ob",
                                       name="pob")
                    for p in range(NKB // 2):  # pairs of k-blocks
                        pss = ppS.tile([128, 2, QC], F32, tag="s", name="pss")
                        if mode == "tiled64":
                            # two concurrent 64-contraction row tiles
                            nc.tensor.matmul(
                                pss[:, 0, :],
                                lhsT=KT[j][0:64, p * 256:p * 256 + 128],
                                rhs=QT[j][0:64, qs:qs + QC],
                                start=True, stop=True)
                            nc.tensor.matmul(
                                pss[:, 1, :],
                                lhsT=KT[j][64:128, p * 256 + 128:p * 256 + 256],
                                rhs=QT[j][64:128, qs:qs + QC],
                                start=True, stop=True)
                        else:
                            for s in range(2):
                                kb = 2 * p + s
                                nc.tensor.matmul(
                                    pss[:, s, :],
                                    lhsT=KT[j][:, kb * 128:(kb + 1) * 128],
                                    rhs=QT[j][:, qs:qs + QC],
                                    start=True, stop=True)
                        es = epool.tile([128, 2, QC], BF16, tag="e", name="es")
                        nc.scalar.activation(
                            out=es, in_=pss,
                            func=mybir.ActivationFunctionType.Exp, scale=0.125)
                        if dbg and j == 0 and qi == 0 and p == 0:
                            nc.sync.dma_start(out=dbg["es"], in_=es)
                        for s in range(2):
                            kb = 2 * p + s
                            first = p == 0 and s == 0
                            last = p == NKB // 2 - 1 and s == 1
                            if mode == "tiled64":
                                nc.tensor.matmul(
                                    poa, lhsT=V[j][0:64, kb, :],
                                    rhs=es[0:64, s, :],
                                    start=first, stop=last,
                                    skip_group_check=True)
                                nc.tensor.matmul(
                                    pob, lhsT=V[j][64:128, kb, :],
                                    rhs=es[64:128, s, :],
                                    start=first, stop=last,
                                    skip_group_check=True)
                            else:
                                nc.tensor.matmul(
                                    poa, lhsT=V[j][:, kb, :],
                                    rhs=es[:, s, :],
                                    start=first, stop=last,
                                    skip_group_check=True)
                    # evacuate (DVE may read only one PSUM operand per op)
                    nc.vector.tensor_copy(out=OT[j][:, qs:qs + QC], in_=poa)
                    if mode == "tiled64":
                        nc.vector.tensor_add(
                            out=OT[j][:, qs:qs + QC],
                            in0=OT[j][:, qs:qs + QC], in1=pob)
                    # reciprocal of sums in place (row 64)
                    nc.vector.reciprocal(
                        out=OT[j][DK:DK + 1, qs:qs + QC],
                        in_=OT[j][DK:DK + 1, qs:qs + QC])
                    if dbg and j == 0 and qi == 0:
                        nc.sync.dma_start(
                            out=dbg["otr"],
                            in_=OT[0][:, 0:QC].bitcast(F32))
                    # broadcast recip across partitions and scale O^T.
                    # partition_broadcast reads PHYSICAL partition 0, so
                    # stage the recip row there via a tiny DMA first.
                    srow = rpool.tile([1, QC], OT_DT, tag="sr", name="srow")
                    nc.sync.dma_start(
                        out=srow, in_=OT[j][DK:DK + 1, qs:qs + QC])
                    rbc = rpool.tile([DK + 1, QC], OT_DT, tag="r", name="rbc")
                    nc.gpsimd.partition_broadcast(rbc, srow, channels=DK + 1)
                    if dbg and j == 0 and qi == 0:
                        nc.sync.dma_start(out=dbg["rbc"], in_=rbc.bitcast(F32))
                    nc.vector.tensor_mul(
                        out=OT[j][0:DK, qs:qs + QC],
                        in0=OT[j][0:DK, qs:qs + QC], in1=rbc[0:DK, :])

            if dbg:
                nc.sync.dma_start(out=dbg["ot"], in_=OT[0].bitcast(F32))

            # ---- Phase C: output projection ----
            for t in range(S // 128):
                c1 = ppA.tile([128, 512], F32, tag="s", name="c1")
                c2 = ppA.tile([128, 256], F32, tag="s", name="c2")
                for j in range(HPC):
                    nc.tensor.matmul(
                        c1, lhsT=OT[j][0:DK, t * 128:(t + 1) * 128],
                        rhs=wosb[:, j, 0:512],
                        start=(j == 0), stop=(j == HPC - 1))
                for j in range(HPC):
                    nc.tensor.matmul(
                        c2, lhsT=OT[j][0:DK, t * 128:(t + 1) * 128],
                        rhs=wosb[:, j, 512:768],
                        start=(j == 0), stop=(j == HPC - 1))
                ot = opool.tile([128, D], F32, tag="o", name="ot")
                nc.vector.tensor_copy(out=ot[:, 0:512], in_=c1)
                nc.vector.tensor_copy(out=ot[:, 512:768], in_=c2)
                nc.sync.dma_start(out=out_d[t * 128:(t + 1) * 128, :], in_=ot)




def _emit_v2(nc, tc, xT_d, wp_d, bp_d, wo_d, out_d, exp_group=4):
    """Per-head pipeline; S^T psum in bf16 when exp_group=4 (2048-wide exp)."""
    import contextlib
    ctx = contextlib.ExitStack()
    with ctx:
        wpool = ctx.enter_context(tc.tile_pool(name="wpool", bufs=1))
        persist = ctx.enter_context(tc.tile_pool(name="persist", bufs=1))
        hpool = ctx.enter_context(tc.tile_pool(name="hpool", bufs=2))
        xpool = ctx.enter_context(tc.tile_pool(name="xpool", bufs=2))
        epool = ctx.enter_context(tc.tile_pool(name="epool", bufs=4))
        rpool = ctx.enter_context(tc.tile_pool(name="rpool", bufs=1))
        opool = ctx.enter_context(tc.tile_pool(name="opool", bufs=2))
        # one shared PSUM pool for S-groups/proj/transposes/phase C
        # (3 slots of 2 banks) + the two O accumulators (1 bank each)
        ppS = ctx.enter_context(tc.tile_pool(name="ppS", bufs=3, space="PSUM"))
        ppO = ctx.enter_context(tc.tile_pool(name="ppO", bufs=1, space="PSUM"))
        ppA = ppS

        SDT = BF16 if exp_group == 4 else F32
        NG = NKB // exp_group

        wsb = wpool.tile([128, HPC, 2, 6, 128], F32R)
        nc.sync.dma_start(out=wsb, in_=wp_d.rearrange("j g c p m -> p j g c m"))
        bsb = wpool.tile([128, HPC, 2], F32)
        nc.sync.dma_start(out=bsb, in_=bp_d)
        wosb = wpool.tile([DK, HPC, D], F32R)
        nc.sync.dma_start(out=wosb, in_=wo_d.rearrange("j d m -> d j m"))
        ident = wpool.tile([128, 128], BF16)
        make_identity(nc, ident)

        OT = [persist.tile([DK + 1, S], OT_DT, tag=f"ot{j}", name=f"ot{j}")
              for j in range(HPC)]

        def emit_c(cqi):
            for t in range(cqi * QC // 128, (cqi + 1) * QC // 128):
                c1 = ppO.tile([128, 512], F32, tag="oa", name="c1")
                c2 = ppO.tile([128, 256], F32, tag="ob", name="c2")
                for jj in range(HPC):
                    nc.tensor.matmul(
                        c1, lhsT=OT[jj][0:DK, t * 128:(t + 1) * 128],
                        rhs=wosb[:, jj, 0:512],
                        start=(jj == 0), stop=(jj == HPC - 1))
                for jj in range(HPC):
                    nc.tensor.matmul(
                        c2, lhsT=OT[jj][0:DK, t * 128:(t + 1) * 128],
                        rhs=wosb[:, jj, 512:768],
                        start=(jj == 0), stop=(jj == HPC - 1))
                ot = opool.tile([128, D], F32, tag="o", name="ot")
                nc.vector.tensor_copy(out=ot[:, 0:512], in_=c1)
                nc.vector.tensor_copy(out=ot[:, 512:768], in_=c2)
                nc.sync.dma_start(
                    out=out_d[t * 128:(t + 1) * 128, :], in_=ot)

        n_xch = S // XCH
        for j in range(HPC):
            # ---- phase A for head j ----
            QT = hpool.tile([128, S], BF16, tag="qt", name="qt")
            KT = hpool.tile([128, S], BF16, tag="kt", name="kt")
            VT = hpool.tile([128, S], BF16, tag="vt", name="vt")
            V = hpool.tile([128, NKB, DK + 1], BF16, tag="v", name="v")
            nc.vector.memset(V[:, :, DK], 1.0)
            for ci in range(n_xch):
                xq = xpool.tile([128, 6, XCH], F32R, tag="x", name="xq")
                nc.sync.dma_start(
                    out=xq,
                    in_=xT_d.rearrange("(c p) q -> p c q", p=128)[
                        :, :, ci * XCH:(ci + 1) * XCH])
                cs = slice(ci * XCH, (ci + 1) * XCH)
                # group 0: (Q | K)
                ps = ppA.tile([128, XCH], F32, tag="s", name="ps")
                for c in range(6):
                    nc.tensor.matmul(
                        ps, lhsT=wsb[:, j, 0, c, :], rhs=xq[:, c, :],
                        start=(c == 0), stop=(c == 5))
                nc.vector.tensor_scalar_add(
                    out=QT[0:64, cs], in0=ps[0:64, :],
                    scalar1=bsb[0:64, j, 0:1])
                nc.vector.tensor_scalar_add(
                    out=KT[64:128, cs], in0=ps[64:128, :],
                    scalar1=bsb[64:128, j, 0:1])
                # group 1: (V | V) duplicated
                ps2 = ppA.tile([128, XCH], F32, tag="s", name="ps2")
                for c in range(6):
                    nc.tensor.matmul(
                        ps2, lhsT=wsb[:, j, 1, c, :], rhs=xq[:, c, :],
                        start=(c == 0), stop=(c == 5))
                nc.vector.tensor_scalar_add(
                    out=VT[:, cs], in0=ps2, scalar1=bsb[:, j, 1:2])
                # V natural layout via PE transposes (chunk's k-blocks)
                for kb in range(ci * XCH // 128, (ci + 1) * XCH // 128):
                    pt = ppA.tile([128, 128], BF16, tag="s", name="pt")
                    nc.tensor.transpose(
                        pt, VT[:, kb * 128:(kb + 1) * 128], ident)
                    nc.vector.tensor_copy(
                        out=V[:, kb, 0:DK], in_=pt[:, 0:DK])
            # duplicate halves: Q lower->upper, K upper->lower
            nc.sync.dma_start(out=QT[64:128, :], in_=QT[0:64, :])
            nc.sync.dma_start(out=KT[0:64, :], in_=KT[64:128, :])

            # ---- phase B for head j ----
            for qi in range(NQC):
                qs = qi * QC
                poa = ppO.tile([DK + 1, QC], F32, tag="oa", name="poa")
                pob = ppO.tile([DK + 1, QC], F32, tag="ob", name="pob")
                for g in range(NG):
                    pss = ppS.tile([128, exp_group, QC], SDT, tag="s",
                                   name="pss")
                    # T0 row-tile: first half of the group's k-blocks;
                    # T8: second half (separate PSUM banks)
                    hg = exp_group // 2
                    for i in range(hg):
                        kb = g * exp_group + i
                        nc.tensor.matmul(
                            pss[:, i, :],
                            lhsT=KT[0:64, kb * 128:(kb + 1) * 128],
                            rhs=QT[0:64, qs:qs + QC],
                            start=True, stop=True)
                    for i in range(hg):
                        kb = g * exp_group + hg + i
                        nc.tensor.matmul(
                            pss[:, hg + i, :],
                            lhsT=KT[64:128, kb * 128:(kb + 1) * 128],
                            rhs=QT[64:128, qs:qs + QC],
                            start=True, stop=True)
                    es = epool.tile([128, exp_group, QC], BF16, tag="e",
                                    name="es")
                    nc.scalar.activation(
                        out=es, in_=pss,
                        func=mybir.ActivationFunctionType.Exp, scale=0.125)
                    for s in range(exp_group):
                        kb = g * exp_group + s
                        first = g == 0 and s == 0
                        last = g == NG - 1 and s == exp_group - 1
                        nc.tensor.matmul(
                            poa, lhsT=V[0:64, kb, :], rhs=es[0:64, s, :],
                            start=first, stop=last, skip_group_check=True)
                        nc.tensor.matmul(
                            pob, lhsT=V[64:128, kb, :], rhs=es[64:128, s, :],
                            start=first, stop=last, skip_group_check=True)
                nc.vector.tensor_copy(out=OT[j][:, qs:qs + QC], in_=poa)
                nc.vector.tensor_add(
                    out=OT[j][:, qs:qs + QC],
                    in0=OT[j][:, qs:qs + QC], in1=pob)
                nc.vector.reciprocal(
                    out=OT[j][DK:DK + 1, qs:qs + QC],
                    in_=OT[j][DK:DK + 1, qs:qs + QC])
                srow = rpool.tile([1, QC], OT_DT, tag="sr", name="srow")
                nc.sync.dma_start(
                    out=srow, in_=OT[j][DK:DK + 1, qs:qs + QC])
                rbc = rpool.tile([DK + 1, QC], OT_DT, tag="r", name="rbc")
                nc.gpsimd.partition_broadcast(rbc, srow, channels=DK + 1)
                nc.vector.tensor_mul(
                    out=OT[j][0:DK, qs:qs + QC],
                    in0=OT[j][0:DK, qs:qs + QC], in1=rbc[0:DK, :])

        # ---- phase C: output projection (borrows psumO slots) ----
        for cqi in range(NQC):
            emit_c(cqi)




# Schraudolph fast-exp on DVE: i16 = round(s*A + B); bitcast to bf16
# approximates exp(0.125*s) with ~3% max rel err. Groups listed in
# DVE_EXP_GROUPS (of the 16 exp-groups per q-chunk) use it; rest use ACT.
SCH_A = 0.125 * np.log2(np.e) * 128.0
SCH_B = 127.0 * 128.0 - 4.8
DVE_EXP_GROUPS = frozenset((1, 4, 7, 10, 13))
# which of the 16 exp-groups per q-chunk use the DVE fast exp, per mode
DVE_PATTERNS = {
    "v3s": frozenset((1, 4, 7, 10, 13)),
    "v3ps": frozenset((1, 4, 7, 10, 13)),
    "v3ps5": frozenset(range(1, 16, 2)),
    "v3ps6": frozenset((1, 2, 4, 5, 7, 8, 10, 11, 13, 14)),
}


def _emit_v3(nc, tc, xT_d, wp_d, bp_d, wo_d, out_d, dve_exp=False,
             no_c=False, no_o=False, no_exp=False, packed=False,
             exp4=False, dve_groups=frozenset(), early=False,
             split_o=False, ot_bf16=False, fast_recip=False, group3=False):
    """v3: full-contraction O matmuls (single accumulator), separate PSUM
    pools per phase, software-pipelined exp->O, and phase A(j+1)/C emission
    interleaved into phase B(j) q-chunks.

    packed (v3p): head 0's V-projection group computes (V0|V1) so head 1
    needs no V group and one transpose serves two heads; head 2 keeps a
    duplicated (V2|V2) group. VT becomes a per-chunk transient tile.

    no_c/no_o/no_exp are timing-only ablations (wrong results)."""
    import contextlib
    ctx = contextlib.ExitStack()
    with ctx:
        wpool = ctx.enter_context(tc.tile_pool(name="wpool", bufs=1))
        otpool = ctx.enter_context(tc.tile_pool(name="otpool", bufs=1))
        hpool = ctx.enter_context(tc.tile_pool(name="hpool", bufs=2))
        xpool = ctx.enter_context(tc.tile_pool(name="xpool", bufs=2))
        epool = ctx.enter_context(tc.tile_pool(name="epool", bufs=4))
        rpool = ctx.enter_context(tc.tile_pool(name="rpool", bufs=1))
        opool = ctx.enter_context(tc.tile_pool(name="opool", bufs=2))
        vtpool = ctx.enter_context(tc.tile_pool(name="vtpool", bufs=2))
        # PSUM budget (8 banks): S-score slots 2x2, O-accum/phase-C 2x1,
        # phase-A proj/transpose 2x1
        assert not (group3 and split_o)
        ppS = ctx.enter_context(tc.tile_pool(name="ppS", bufs=2, space="PSUM"))
        ppO = ctx.enter_context(tc.tile_pool(
            name="ppO", bufs=1 if group3 else 2, space="PSUM"))
        ppA = ctx.enter_context(tc.tile_pool(
            name="ppA", bufs=1 if (split_o or group3) else 2, space="PSUM"))
        ppOb = ctx.enter_context(
            tc.tile_pool(name="ppOb", bufs=1, space="PSUM")) if split_o \
            else None
        # k-blocks per exp-group: 3 needs 3-bank score slots (6 banks), so
        # phase A/C share the 2 remaining single-bank pools
        GW = 3 if group3 else (4 if exp4 else 2)
        exp_groups = [tuple(range(s, min(s + GW, NKB)))
                      for s in range(0, NKB, GW)]

        wsb = wpool.tile([128, HPC, 2, 6, 128], F32R)
        nc.sync.dma_start(out=wsb, in_=wp_d.rearrange("j g c p m -> p j g c m"))
        bsb = wpool.tile([128, HPC, 2], F32)
        nc.sync.dma_start(out=bsb, in_=bp_d)
        if ot_bf16:
            wosf = wpool.tile([DK, HPC, D], F32R, tag="wof", name="wosf")
            nc.sync.dma_start(out=wosf, in_=wo_d.rearrange("j d m -> d j m"))
            wosb = wpool.tile([DK, HPC, D], BF16, tag="wo", name="wosb")
            nc.vector.tensor_copy(out=wosb, in_=wosf)
        else:
            wosb = wpool.tile([DK, HPC, D], F32R)
            nc.sync.dma_start(out=wosb, in_=wo_d.rearrange("j d m -> d j m"))
        ident = wpool.tile([128, 128], BF16)
        make_identity(nc, ident)

        odt = BF16 if ot_bf16 else OT_DT
        OT = [otpool.tile([DK + 1, S], odt, tag=f"ot{j}", name=f"ot{j}")
              for j in range(HPC)]

        es_const = None
        if no_exp:
            es_const = wpool.tile([128, 2, QC], BF16, tag="ec", name="ec")
            nc.vector.memset(es_const, 0.001)

        n_xch = S // XCH

        def alloc_head(j):
            t = {
                "QT": hpool.tile([128, S], BF16, tag="qt", name="qt"),
                "KT": hpool.tile([128, S], BF16, tag="kt", name="kt"),
                "V": hpool.tile([128, NKB, DK + 1], BF16, tag="v", name="v"),
            }
            if not packed:
                t["VT"] = hpool.tile([128, S], BF16, tag="vt", name="vt")
            nc.vector.memset(t["V"][:, :, DK], 1.0)
            return t

        def emit_A(j, t, ci, v_dsts=None):
            QT, KT = t["QT"], t["KT"]
            if v_dsts is None:
                v_dsts = [(t["V"], 0)]
            xq = xpool.tile([128, 6, XCH], F32R, tag="x", name="xq")
            nc.sync.dma_start(
                out=xq,
                in_=xT_d.rearrange("(c p) q -> p c q", p=128)[
                    :, :, ci * XCH:(ci + 1) * XCH])
            cs = slice(ci * XCH, (ci + 1) * XCH)
            # group 0: (Q | K)
            ps = ppA.tile([128, XCH], F32, tag="a", name="ps")
            for c in range(6):
                nc.tensor.matmul(
                    ps, lhsT=wsb[:, j, 0, c, :], rhs=xq[:, c, :],
                    start=(c == 0), stop=(c == 5))
            nc.vector.tensor_scalar_add(
                out=QT[0:64, cs], in0=ps[0:64, :], scalar1=bsb[0:64, j, 0:1])
            nc.vector.tensor_scalar_add(
                out=KT[64:128, cs], in0=ps[64:128, :],
                scalar1=bsb[64:128, j, 0:1])
            # duplicate halves per chunk (Q lower->upper, K upper->lower)
            nc.sync.dma_start(out=QT[64:128, cs], in_=QT[0:64, cs])
            nc.sync.dma_start(out=KT[0:64, cs], in_=KT[64:128, cs])
            if not v_dsts:
                return
            # group 1: V halves (either (Vj|Vj) dup'd or (Vj|Vj+1) packed)
            ps2 = ppA.tile([128, XCH], F32, tag="a", name="ps2")
            for c in range(6):
                nc.tensor.matmul(
                    ps2, lhsT=wsb[:, j, 1, c, :], rhs=xq[:, c, :],
                    start=(c == 0), stop=(c == 5))
            if packed:
                VT = vtpool.tile([128, XCH], BF16, tag="vt", name="vtc")
                vcs = slice(0, XCH)
            else:
                VT = t["VT"]
                vcs = cs
            nc.vector.tensor_scalar_add(
                out=VT[:, vcs], in0=ps2, scalar1=bsb[:, j, 1:2])
            # V natural layout via PE transposes; all 4 blocks land in one
            # psum tile so one strided DVE copy evacuates them per head
            pt4 = ppA.tile([128, XCH // 128, 128], BF16, tag="a", name="pt4")
            for kk in range(XCH // 128):
                lo = (0 if packed else ci * XCH) + kk * 128
                nc.tensor.transpose(pt4[:, kk, :], VT[:, lo:lo + 128], ident)
            kb0 = ci * XCH // 128
            for Vt, off in v_dsts:
                nc.vector.tensor_copy(
                    out=Vt[:, kb0:kb0 + XCH // 128, 0:DK],
                    in_=pt4[:, :, off:off + DK])

        def b_chunk_gen(j, t, qi):
            QT, KT, V = t["QT"], t["KT"], t["V"]
            qs = qi * QC
            poa = ppO.tile([DK + 1, QC], F32, tag="oa", name="poa")
            pob = ppOb.tile([DK + 1, QC], F32, tag="ob", name="pob") \
                if split_o else None

            def emit_O(es, kbs):
                if no_o:
                    if kbs[0] == 0:
                        nc.vector.memset(poa, 1.0)
                    return
                for s, kb in enumerate(kbs):
                    st, sp = (kb == 0), (kb == NKB - 1)
                    if split_o:
                        # half-contraction pair on disjoint row groups: the
                        # two matmuls overlap on the PE array
                        nc.tensor.matmul(
                            poa, lhsT=V[0:64, kb, :], rhs=es[0:64, s, :],
                            start=st, stop=sp, skip_group_check=True)
                        nc.tensor.matmul(
                            pob, lhsT=V[64:128, kb, :], rhs=es[64:128, s, :],
                            start=st, stop=sp, skip_group_check=True)
                    else:
                        nc.tensor.matmul(
                            poa, lhsT=V[:, kb, :], rhs=es[:, s, :],
                            start=st, stop=sp, skip_group_check=True)

            pend = None
            for gi, kbs in enumerate(exp_groups):
                pss = ppS.tile([128, len(kbs), QC], FP16 if exp4 else F32,
                               tag="s", name="pss")
                # alternate row halves so consecutive matmuls hit disjoint
                # row groups (also across group boundaries)
                for idx, kb in enumerate(kbs):
                    lo = 64 * ((idx + gi * GW) % 2)
                    nc.tensor.matmul(
                        pss[:, idx, :],
                        lhsT=KT[lo:lo + 64, kb * 128:(kb + 1) * 128],
                        rhs=QT[lo:lo + 64, qs:qs + QC],
                        start=True, stop=True)
                if no_exp:
                    es = es_const
                elif (dve_exp and gi in DVE_EXP_GROUPS) or gi in dve_groups:
                    esi = epool.tile([128, len(kbs), QC], I16, tag="e",
                                     name="esi")
                    nc.vector.tensor_scalar(
                        out=esi, in0=pss, scalar1=SCH_A, scalar2=SCH_B,
                        op0=mybir.AluOpType.mult, op1=mybir.AluOpType.add)
                    es = esi.bitcast(BF16)
                else:
                    es = epool.tile([128, len(kbs), QC], BF16, tag="e",
                                    name="es")
                    nc.scalar.activation(
                        out=es, in_=pss,
                        func=mybir.ActivationFunctionType.Exp, scale=0.125)
                if pend is not None:
                    emit_O(*pend)
                pend = (es, kbs)
                yield
            emit_O(*pend)

            nc.vector.tensor_copy(out=OT[j][:, qs:qs + QC], in_=poa)
            if split_o:
                # DVE may read only one PSUM operand per op
                nc.vector.tensor_add(
                    out=OT[j][:, qs:qs + QC],
                    in0=OT[j][:, qs:qs + QC], in1=pob)
            if fast_recip:
                rio = OT[j][DK:DK + 1, qs:qs + QC].bitcast(F32)
                rscr = rpool.tile([1, QC], F32, tag="rs", name="rscr")
                nc.vector.reciprocal_approx_accurate(
                    out=rio, in_=rio, scratch=rscr)
            else:
                nc.vector.reciprocal(
                    out=OT[j][DK:DK + 1, qs:qs + QC],
                    in_=OT[j][DK:DK + 1, qs:qs + QC])
            srow = rpool.tile([1, QC], odt, tag="sr", name="srow")
            nc.sync.dma_start(out=srow, in_=OT[j][DK:DK + 1, qs:qs + QC])
            rbc = rpool.tile([DK + 1, QC], odt, tag="r", name="rbc")
            nc.gpsimd.partition_broadcast(rbc, srow, channels=DK + 1)
            nc.vector.tensor_mul(
                out=OT[j][0:DK, qs:qs + QC],
                in0=OT[j][0:DK, qs:qs + QC], in1=rbc[0:DK, :])

        def emit_B_chunk(j, t, qi):
            for _ in b_chunk_gen(j, t, qi):
                pass

        def emit_C_range(qi):
            if no_c:
                return
            for ti in range(qi * QC // 128, (qi + 1) * QC // 128):
                c1 = (ppA if group3 else ppO).tile(
                    [128, 512], F32, tag="a" if group3 else "oa", name="c1")
                for jj in range(HPC):
                    nc.tensor.matmul(
                        c1, lhsT=OT[jj][0:DK, ti * 128:(ti + 1) * 128],
                        rhs=wosb[:, jj, 0:512],
                        start=(jj == 0), stop=(jj == HPC - 1))
                c2 = ppO.tile([128, 256], F32, tag="oa", name="c2")
                for jj in range(HPC):
                    nc.tensor.matmul(
                        c2, lhsT=OT[jj][0:DK, ti * 128:(ti + 1) * 128],
                        rhs=wosb[:, jj, 512:768],
                        start=(jj == 0), stop=(jj == HPC - 1))
                ot = opool.tile([128, D], F32, tag="o", name="ot")
                nc.vector.tensor_copy(out=ot[:, 0:512], in_=c1)
                nc.vector.tensor_copy(out=ot[:, 512:768], in_=c2)
                nc.sync.dma_start(
                    out=out_d[ti * 128:(ti + 1) * 128, :], in_=ot)

        # prologue: head 0 phase A, optionally with B(0, qi=0) groups
        # spliced in (each A chunk readies 4 k-blocks = 2 exp groups,
        # lagging one chunk)
        heads = [alloc_head(0)]
        if packed:
            heads.append(alloc_head(1))
        pro_v = [(heads[0]["V"], 0), (heads[1]["V"], 64)] if packed else None
        b0 = b_chunk_gen(0, heads[0], 0) if early else None
        done = 0
        for ci in range(n_xch):
            emit_A(0, heads[0], ci, v_dsts=pro_v)
            if early:
                # emit B(0,0) groups whose k-blocks come from chunks < ci
                target = sum(1 for kbs in exp_groups if kbs[-1] < 4 * ci)
                while done < min(target, len(exp_groups)):
                    next(b0)
                    done += 1
        if early:
            for _ in b0:
                pass
        for j in range(HPC):
            t = heads[j]
            if j + 1 < HPC and len(heads) == j + 1:
                heads.append(alloc_head(j + 1))
            q0 = 1 if (early and j == 0) else 0
            for qi in range(q0, NQC):
                emit_B_chunk(j, t, qi)
                if j + 1 < HPC:
                    if packed:
                        v_dsts = [] if j == 0 else [(heads[2]["V"], 0)]
                        emit_A(j + 1, heads[j + 1], qi - q0, v_dsts=v_dsts)
                    else:
                        emit_A(j + 1, heads[j + 1], qi - q0)
                elif qi > 0:
                    # lag C by one q-chunk so its psum-slot reuse and the
                    # OT normalization latency chain decouple from B's
                    # pipeline
                    emit_C_range(qi - 1)
            if j + 1 < HPC:
                for ci in range(NQC - q0, NQC):
                    if packed:
                        v_dsts = [] if j == 0 else [(heads[2]["V"], 0)]
                        emit_A(j + 1, heads[j + 1], ci, v_dsts=v_dsts)
                    else:
                        emit_A(j + 1, heads[j + 1], ci)
        emit_C_range(NQC - 1)


def _emit_mb(nc, tc, out_d, paired=True, n_slots=1024):
    """Microbenchmark: 2048 64-contraction matmuls (N=512), either as
    row-half pairs writing separate banks (paired) or serial
    full-contraction (unpaired: 2048 full-row MMs). Measures whether
    row-tiled MMs overlap on this hardware."""
    import contextlib
    ctx = contextlib.ExitStack()
    with ctx:
        pool = ctx.enter_context(tc.tile_pool(name="mb", bufs=1))
        opool = ctx.enter_context(tc.tile_pool(name="mbo", bufs=2))
        pp = ctx.enter_context(tc.tile_pool(name="mbp", bufs=2, space="PSUM"))
        KT = pool.tile([128, S], BF16, tag="kt", name="kt")
        QT = pool.tile([128, QC], BF16, tag="qt", name="qt")
        nc.vector.memset(KT, 0.01)
        nc.vector.memset(QT, 0.01)
        NACC = 64  # matmuls accumulated per psum bank before evacuation
        for rep in range(n_slots // NACC):
            ps = pp.tile([128, 2, QC], F32, tag="s", name="ps")
            for i in range(NACC):
                kb = (rep * NACC + i) % NKB
                st, sp = (i == 0), (i == NACC - 1)
                if paired:
                    nc.tensor.matmul(
                        ps[:, 0, :], lhsT=KT[0:64, kb * 128:(kb + 1) * 128],
                        rhs=QT[0:64, :], start=st, stop=sp,
                        skip_group_check=True)
                    nc.tensor.matmul(
                        ps[:, 1, :], lhsT=KT[64:128, kb * 128:(kb + 1) * 128],
                        rhs=QT[64:128, :], start=st, stop=sp,
                        skip_group_check=True)
                else:
                    nc.tensor.matmul(
                        ps[:, 0, :], lhsT=KT[:, kb * 128:(kb + 1) * 128],
                        rhs=QT, start=st, stop=sp, skip_group_check=True)
                    nc.tensor.matmul(
                        ps[:, 1, :], lhsT=KT[:, kb * 128:(kb + 1) * 128],
                        rhs=QT, start=st, stop=sp, skip_group_check=True)
            ot = opool.tile([128, 2, QC], F32, tag="o", name="ot")
            nc.vector.tensor_copy(out=ot, in_=ps)
            if rep == 0:
                nc.sync.dma_start(
                    out=out_d[0:128, 0:QC], in_=ot[:, 0, :])


# ---------------------------------------------------------------------------
# host side
# ---------------------------------------------------------------------------

KERNEL_MODE = "v3peg"


def shard_inputs(x, Wq, bq, Wk, bk, Wv, bv, Wo, bo, mode=None):
    """Build the 8 per-core input maps."""
    mode = mode or KERNEL_MODE
    if mode.startswith("v5") or (mode.startswith("v3") and "p" in mode):
        return shard_inputs_v3p(x, Wq, bq, Wk, bk, Wv, bv, Wo, bo)
    if mode.startswith("v2") or mode.startswith("v3"):
        return shard_inputs_v2(x, Wq, bq, Wk, bk, Wv, bv, Wo, bo)
    return shard_inputs_v1(x, Wq, bq, Wk, bk, Wv, bv, Wo, bo)


def shard_inputs_v3p(x, Wq, bq, Wk, bk, Wv, bv, Wo, bo):
    """v3p packing: wp[0,1] = (V0|V1), wp[1,1] unused, wp[2,1] = (V2|V2)."""
    x = np.asarray(x, np.float32)
    Wq, Wk, Wv = (np.asarray(a, np.float32) for a in (Wq, Wk, Wv))
    bq, bk, bv = (np.asarray(a, np.float32) for a in (bq, bk, bv))
    Wo = np.asarray(Wo, np.float32)
    in_maps = []
    for c in range(N_CORES):
        b, g = divmod(c, 4)
        heads = [3 * g + j for j in range(HPC)]
        wp = np.zeros((HPC, 2, 6, 128, 128), np.float32)
        bp = np.zeros((128, HPC, 2), np.float32)
        wo = np.empty((HPC, DK, D), np.float32)
        for j, h in enumerate(heads):
            sl = slice(64 * h, 64 * h + 64)
            wp[j, 0, :, :, 0:64] = Wq[sl].T.reshape(6, 128, 64)
            wp[j, 0, :, :, 64:128] = Wk[sl].T.reshape(6, 128, 64)
            bp[0:64, j, 0] = bq[sl]
            bp[64:128, j, 0] = bk[sl]
            wo[j] = Wo[:, sl].T
        # V groups: head0 -> (V0|V1); head2 -> (V2|V2)
        h0, h1, h2 = heads
        for (jj, half, h) in ((0, 0, h0), (0, 1, h1), (2, 0, h2), (2, 1, h2)):
            sl = slice(64 * h, 64 * h + 64)
            wp[jj, 1, :, :, 64 * half:64 * half + 64] = \
                Wv[sl].T.reshape(6, 128, 64)
            bp[64 * half:64 * half + 64, jj, 1] = bv[sl]
        in_maps.append({
            "xT": np.ascontiguousarray(x[b].T),
            "wp": wp, "bp": bp, "wo": wo,
        })
    return in_maps


def shard_inputs_v2(x, Wq, bq, Wk, bk, Wv, bv, Wo, bo):
    x = np.asarray(x, np.float32)
    Wq, Wk, Wv = (np.asarray(a, np.float32) for a in (Wq, Wk, Wv))
    bq, bk, bv = (np.asarray(a, np.float32) for a in (bq, bk, bv))
    Wo = np.asarray(Wo, np.float32)
    in_maps = []
    for c in range(N_CORES):
        b, g = divmod(c, 4)
        heads = [3 * g + j for j in range(HPC)]
        wp = np.empty((HPC, 2, 6, 128, 128), np.float32)
        bp = np.zeros((128, HPC, 2), np.float32)
        wo = np.empty((HPC, DK, D), np.float32)
        for j, h in enumerate(heads):
            sl = slice(64 * h, 64 * h + 64)
            wp[j, 0, :, :, 0:64] = Wq[sl].T.reshape(6, 128, 64)
            wp[j, 0, :, :, 64:128] = Wk[sl].T.reshape(6, 128, 64)
            wp[j, 1, :, :, 0:64] = Wv[sl].T.reshape(6, 128, 64)
            wp[j, 1, :, :, 64:128] = Wv[sl].T.reshape(6, 128, 64)
            bp[0:64, j, 0] = bq[sl]
            bp[64:128, j, 0] = bk[sl]
            bp[0:64, j, 1] = bv[sl]
            bp[64:128, j, 1] = bv[sl]
            wo[j] = Wo[:, sl].T
        in_maps.append({
            "xT": np.ascontiguousarray(x[b].T),
            "wp": wp, "bp": bp, "wo": wo,
        })
    return in_maps


def shard_inputs_v1(x, Wq, bq, Wk, bk, Wv, bv, Wo, bo):
    """Build the 8 per-core input maps."""
    x = np.asarray(x, np.float32)
    Ws = {0: np.asarray(Wq, np.float32), 1: np.asarray(Wk, np.float32),
          2: np.asarray(Wv, np.float32)}
    bs = {0: np.asarray(bq, np.float32), 1: np.asarray(bk, np.float32),
          2: np.asarray(bv, np.float32)}
    Wo = np.asarray(Wo, np.float32)
    in_maps = []
    for c in range(N_CORES):
        b, g = divmod(c, 4)
        heads = [3 * g + j for j in range(HPC)]
        wp = np.empty((5, 6, 128, 128), np.float32)
        bp = np.zeros((128, 5), np.float32)
        for gi, (mA, mB) in enumerate(PROJ_GROUPS):
            for half, (j, kind) in ((0, mA), (1, mB)):
                h = heads[j]
                Wh = Ws[kind][64 * h:64 * h + 64, :]       # [64, 768]
                chunks = Wh.T.reshape(6, 128, 64)          # [c, p, 64]
                wp[gi, :, :, half * 64:half * 64 + 64] = chunks
                bp[half * 64:half * 64 + 64, gi] = bs[kind][64 * h:64 * h + 64]
        wo = np.empty((HPC, DK, D), np.float32)
        for j in range(HPC):
            h = heads[j]
            wo[j] = Wo[:, 64 * h:64 * h + 64].T
        in_maps.append({
            "xT": np.ascontiguousarray(x[b].T),
            "wp": wp, "bp": bp, "wo": wo,
        })
    return in_maps


def assemble_output(parts, bo):
    out = np.empty((B, S, D), np.float32)
    for b in range(B):
        acc = parts[4 * b]["out"].astype(np.float32).copy()
        for c in range(4 * b + 1, 4 * b + 4):
            acc += parts[c]["out"]
        out[b] = acc + np.asarray(bo, np.float32)[None, :]
    return out


_RUNNER = None


def _make_runner(nc):
    """Reusable PJRT runner (mirrors bass2jax.run_bass_via_pjrt multi-core)."""
    import jax
    import jax.numpy as jnp
    from jax.experimental.shard_map import shard_map
    from jax.sharding import Mesh, PartitionSpec
    from concourse import bass2jax

    bass2jax.install_neuronx_cc_hook()

    partition_name = (nc.partition_id_tensor.name
                      if nc.partition_id_tensor else None)
    in_names, out_names, out_avals = [], [], []
    for alloc in nc.m.functions[0].allocations:
        if not isinstance(alloc, mybir.MemoryLocationSet):
            continue
        name = alloc.memorylocations[0].name
        if alloc.kind == "ExternalInput":
            if name != partition_name:
                in_names.append(name)
        elif alloc.kind == "ExternalOutput":
            out_names.append(name)
            out_avals.append(jax.core.ShapedArray(
                tuple(alloc.tensor_shape), mybir.dt.np(alloc.dtype)))
    n_params = len(in_names)
    n_outs = len(out_names)
    all_in_names = list(in_names) + list(out_names)
    if partition_name is not None:
        all_in_names.append(partition_name)
    donate = tuple(range(n_params, n_params + n_outs))

    def _body(*args):
        operands = list(args)
        if partition_name is not None:
            operands.append(bass2jax.partition_id_tensor())
        outs = bass2jax._bass_exec_p.bind(
            *operands,
            out_avals=tuple(out_avals),
            in_names=tuple(all_in_names),
            out_names=tuple(out_names),
            lowering_input_output_aliases=(),
            sim_require_finite=True,
            sim_require_nnan=True,
            nc=nc,
        )
        return tuple(outs)

    devices = jax.devices()[:N_CORES]
    mesh = Mesh(np.asarray(devices), ("core",))
    in_specs = (PartitionSpec("core"),) * (n_params + n_outs)
    out_specs = (PartitionSpec("core"),) * n_outs
    sharded = jax.jit(
        shard_map(_body, mesh=mesh, in_specs=in_specs, out_specs=out_specs,
                  check_rep=False),
        donate_argnums=donate, keep_unused=True)

    def run(in_maps):
        per_core = [[np.asarray(m[name]) for name in in_names]
                    for m in in_maps]
        concat_in = [np.concatenate([per_core[c][i] for c in range(N_CORES)],
                                    axis=0) for i in range(n_params)]
        zeros = [np.zeros((N_CORES * av.shape[0], *av.shape[1:]), av.dtype)
                 for av in out_avals]
        outs = sharded(*concat_in, *zeros)
        return [
            {name: np.asarray(outs[i]).reshape(N_CORES, *out_avals[i].shape)[c]
             for i, name in enumerate(out_names)}
            for c in range(N_CORES)
        ]

    run.sharded = sharded
    run.in_names = in_names
    run.out_names = out_names
    run.out_avals = out_avals
    run.n_params = n_params
    return run


def get_runner():
    global _RUNNER
    if _RUNNER is None:
        nc = build_program()
        _RUNNER = _make_runner(nc)
    return _RUNNER


def kernel(x, Wq, bq, Wk, bk, Wv, bv, Wo, bo):
    run = get_runner()
    in_maps = shard_inputs(x, Wq, bq, Wk, bk, Wv, bv, Wo, bo)
    parts = run(in_maps)
    return assemble_output(parts, bo)

